# revision 2
# baseline (speedup 1.0000x reference)
"""AttnBlock (GroupNorm + single-head spatial attention + proj + residual)
on 8 Trainium2 NeuronCores via Bass/Tile.

Sharding: batch b=4 -> 4 samples x 2 cores each. Each core receives its
sample's x with its query-half columns rotated to the front (attention is
permutation-invariant over key positions), computes GroupNorm + k + v for
the full sample (redundant with its pair core) and q/attention/proj for its
2048 query positions. No cross-core communication.
"""

import numpy as np
import ml_dtypes

import concourse.bass as bass
import concourse.tile as tile
import concourse.mybir as mybir
from concourse.bass_utils import run_bass_kernel_spmd
from concourse.vector_clock import ScopedClock, VectorClock
from concourse.tile_scheduler import N_PROCS

# ---------------------------------------------------------------- constants
B, C, H, W = 4, 512, 64, 64
HW = H * W            # 4096
P = 128
NCO = C // P          # 4 channel chunks of 128
G = 32                # groups
IHALF = HW // 2       # 2048 query columns per core
IB = 512              # i-block width
NIB = IHALF // IB     # 4
JBLK = 512            # column block for GN/qkv phases
NJB = HW // JBLK      # 8
NJC = HW // P         # 32 j-chunks of 128
EPS = 1e-6
SCALE = float(1.0 / np.sqrt(C))
F32 = mybir.dt.float32
BF16 = mybir.dt.bfloat16


# ------------------------------------------------- walrus single-wait fixes
class _TileContextFix(tile.TileContext):
    """TileContext whose tail drain splits sem waits across NOPs.

    The walrus build here rejects instructions carrying more than one sync
    wait ("Too many sync wait commands"), so the stock tail drain (one wait
    per outstanding proc) cannot codegen. Emit one single-wait NOP per proc
    before a wait-free drain.
    """

    def _drain_and_barrier(self, tick_clock, wait_clock):
        gc = tick_clock.global_clock
        for p in range(N_PROCS):
            if gc[p] == 0:
                continue
            partial = VectorClock([gc[q] if q == p else 0 for q in range(N_PROCS)])
            nop_inst = self.nc.sync.nop(nofuse=True, hint=f"tail_wait_{p}")
            wait_clock.add_sem_waits(nop_inst.ins, ScopedClock({None: partial}))
        self.nc.sync.drain()
        self.nc.all_engine_barrier()
        assert self.sems is not None
        popped = self.nc._tile_sem_poison_stack.pop()
        assert popped is self._sem_poison
        self.nc.clear_and_free_semaphores(list(self.sems.allocated().values()))
        self.nc.all_engine_barrier()


def _split_multi_waits(nc):
    """Split any instruction with N>1 sync waits into N-1 single-wait NOPs
    prepended on the same engine (same stream -> same ordering; sems are
    monotonic so waiting earlier is safe)."""
    fn = nc.m.functions[0]
    n_split = 0
    for bb in fn.blocks:
        insts = list(bb.instructions)
        out = []
        for inst in insts:
            si = inst.sync_info
            if si is not None and si.on_wait and len(si.on_wait) > 1:
                waits = list(si.on_wait)
                for w in waits[:-1]:
                    nop = mybir.InstNoOp(
                        name=nc.get_next_instruction_name(),
                        engine=inst.engine,
                        sync_info=mybir.SyncInfo(on_wait=[w], on_update=[]),
                        bass_nofuse=True,
                        ins=[],
                        outs=[],
                    )
                    out.append(nop)
                    n_split += 1
                inst.sync_info = mybir.SyncInfo(
                    on_wait=[waits[-1]], on_update=list(si.on_update or [])
                )
            out.append(inst)
        if len(out) != len(insts):
            bb.instructions[:] = out
    return n_split


# ------------------------------------------------------------- the kernel
def build_bass():
    nc = bass.Bass("TRN2", target_bir_lowering=False, debug=False, num_devices=8)

    x_d = nc.dram_tensor("x", [C, HW], F32, kind="ExternalInput")
    wqt_d = nc.dram_tensor("wqt", [C, C], BF16, kind="ExternalInput")
    wkt_d = nc.dram_tensor("wkt", [C, C], BF16, kind="ExternalInput")
    wvt_d = nc.dram_tensor("wvt", [C, C], BF16, kind="ExternalInput")
    wpt_d = nc.dram_tensor("wpt", [C, C], BF16, kind="ExternalInput")
    bq_d = nc.dram_tensor("bq", [P, NCO], F32, kind="ExternalInput")
    bk_d = nc.dram_tensor("bk", [P, NCO], F32, kind="ExternalInput")
    bp_d = nc.dram_tensor("bp", [P, NCO], F32, kind="ExternalInput")
    bvb_d = nc.dram_tensor("bvb", [P, C], F32, kind="ExternalInput")
    gns_d = nc.dram_tensor("gns", [P, NCO], F32, kind="ExternalInput")
    gnb_d = nc.dram_tensor("gnb", [P, NCO], F32, kind="ExternalInput")
    aggm_d = nc.dram_tensor("aggm", [P, 8], F32, kind="ExternalInput")
    bcm_d = nc.dram_tensor("bcm", [8, P], F32, kind="ExternalInput")
    out_d = nc.dram_tensor("out", [C, IHALF], F32, kind="ExternalOutput")

    x_r = x_d.ap().rearrange("(co p) j -> p co j", p=P)        # [128,4,4096]
    out_r = out_d.ap().rearrange("(co p) i -> p co i", p=P)    # [128,4,2048]

    with _TileContextFix(nc) as tc:
        with (
            tc.tile_pool(name="consts", bufs=1) as consts,
            tc.tile_pool(name="xbf", bufs=1) as xbf,
            tc.tile_pool(name="blk", bufs=3) as blk,
            tc.tile_pool(name="hnp", bufs=3) as hnp,
            tc.tile_pool(name="kqv", bufs=1) as kqv,
            tc.tile_pool(name="stat", bufs=1) as stat,
            tc.tile_pool(name="expp", bufs=6) as expp,
            tc.tile_pool(name="usb", bufs=2) as usb,
            tc.tile_pool(name="drp", bufs=2) as drp,
            tc.tile_pool(name="osb", bufs=2) as osb,
            tc.tile_pool(name="psA", bufs=3, space="PSUM") as psA,
            tc.tile_pool(name="psU", bufs=4, space="PSUM") as psU,
            tc.tile_pool(name="psD", bufs=1, space="PSUM") as psD,
        ):
            # ---------------- constants
            wqt_sb = consts.tile([P, NCO, C], BF16)
            nc.sync.dma_start(wqt_sb[:], wqt_d.ap().rearrange("(ci p) o -> p ci o", p=P))
            wkt_sb = consts.tile([P, NCO, C], BF16)
            nc.sync.dma_start(wkt_sb[:], wkt_d.ap().rearrange("(ci p) o -> p ci o", p=P))
            wvt_sb = consts.tile([P, NCO, C], BF16)
            nc.sync.dma_start(wvt_sb[:], wvt_d.ap().rearrange("(ci p) o -> p ci o", p=P))
            wpt_sb = consts.tile([P, NCO, C], BF16)
            nc.sync.dma_start(wpt_sb[:], wpt_d.ap().rearrange("(ci p) o -> p ci o", p=P))
            bq_sb = consts.tile([P, NCO], F32)
            nc.sync.dma_start(bq_sb[:], bq_d.ap())
            bk_sb = consts.tile([P, NCO], F32)
            nc.sync.dma_start(bk_sb[:], bk_d.ap())
            bp_sb = consts.tile([P, NCO], F32)
            nc.sync.dma_start(bp_sb[:], bp_d.ap())
            bvb_sb = consts.tile([P, C], F32)
            nc.sync.dma_start(bvb_sb[:], bvb_d.ap())
            gns_sb = consts.tile([P, NCO], F32)
            nc.sync.dma_start(gns_sb[:], gns_d.ap())
            gnb_sb = consts.tile([P, NCO], F32)
            nc.sync.dma_start(gnb_sb[:], gnb_d.ap())
            aggm_sb = consts.tile([P, 8], F32)
            nc.sync.dma_start(aggm_sb[:], aggm_d.ap())
            bcm_sb = consts.tile([8, P], F32)
            nc.sync.dma_start(bcm_sb[:], bcm_d.ap())
            ones_bf = consts.tile([P, P], BF16)
            nc.vector.memset(ones_bf[:], 1.0)
            eps_sb = consts.tile([8, 1], F32)
            nc.vector.memset(eps_sb[:], EPS)

            x_bf = xbf.tile([P, NCO, HW], BF16)
            stats = stat.tile([P, NCO, NJB, 6], F32)
            mv = stat.tile([P, NCO, 2], F32)

            # ---------------- phase 1: load x, cast to bf16, per-channel stats
            for jb in range(NJB):
                js, je = jb * JBLK, (jb + 1) * JBLK
                x_blk = blk.tile([P, NCO, JBLK], F32, tag="xblk")
                nc.sync.dma_start(x_blk[:], x_r[:, :, js:je])
                nc.vector.tensor_copy(x_bf[:, :, js:je], x_blk[:])
                for co in range(NCO):
                    nc.vector.bn_stats(stats[:, co, jb, :], x_blk[:, co, :])

            # ---------------- phase 3: group stats -> per-channel affine A, B
            for co in range(NCO):
                nc.vector.bn_aggr(mv[:, co, :], stats[:, co, :, :])
            m2 = stat.tile([P, NCO], F32)
            nc.vector.tensor_mul(m2[:], mv[:, :, 0], mv[:, :, 0])
            nc.vector.tensor_add(mv[:, :, 1], mv[:, :, 1], m2[:])  # E[x^2]
            ps_s = psA.tile([P, IB], F32, tag="ps")
            nc.tensor.matmul(
                ps_s[:8, : NCO * 2],
                aggm_sb[:],
                mv[:].rearrange("p co s -> p (co s)"),
                start=True, stop=True,
            )
            grp = stat.tile([8, NCO, 2], F32)
            nc.vector.tensor_copy(grp[:], ps_s[:8, : NCO * 2])
            g2 = stat.tile([8, NCO], F32)
            nc.vector.tensor_mul(g2[:], grp[:, :, 0], grp[:, :, 0])
            nc.vector.tensor_tensor(
                grp[:, :, 1], grp[:, :, 1], g2[:], mybir.AluOpType.subtract
            )  # var_g
            nc.scalar.activation(
                grp[:, :, 1], grp[:, :, 1], mybir.ActivationFunctionType.Sqrt,
                bias=eps_sb[:], scale=1.0,
            )
            nc.vector.reciprocal(grp[:, :, 1], grp[:, :, 1])  # rstd_g
            ps_b = psA.tile([P, IB], F32, tag="ps")
            nc.tensor.matmul(
                ps_b[:, : NCO * 2],
                bcm_sb[:],
                grp[:].rearrange("g co s -> g (co s)"),
                start=True, stop=True,
            )
            mvb = stat.tile([P, NCO, 2], F32)  # per-channel (mean_g, rstd_g)
            nc.vector.tensor_copy(mvb[:], ps_b[:, : NCO * 2])
            A = stat.tile([P, NCO], F32)
            nc.vector.tensor_mul(A[:], mvb[:, :, 1], gns_sb[:])
            t2 = stat.tile([P, NCO], F32)
            nc.vector.tensor_mul(t2[:], mvb[:, :, 0], A[:])
            Bc = stat.tile([P, NCO], F32)
            nc.vector.tensor_tensor(Bc[:], gnb_sb[:], t2[:], mybir.AluOpType.subtract)

            # ---------------- phase 2: hn blocks -> k, vT, q
            k_sb = kqv.tile([P, NCO, HW], BF16)
            q_sb = kqv.tile([P, NCO, IHALF], BF16)
            vT_sb = kqv.tile([P, NJC, C], BF16)
            for jb in range(NJB):
                js, je = jb * JBLK, (jb + 1) * JBLK
                hn_blk = hnp.tile([P, NCO, JBLK], BF16, tag="hn")
                for co in range(NCO):
                    nc.vector.tensor_scalar(
                        hn_blk[:, co, :], x_bf[:, co, js:je],
                        A[:, co : co + 1], Bc[:, co : co + 1],
                        op0=mybir.AluOpType.mult, op1=mybir.AluOpType.add,
                    )
                # k = wk @ hn  (channel chunks on psum partitions)
                for o in range(NCO):
                    kps = psA.tile([P, IB], F32, tag="ps")
                    for ci in range(NCO):
                        nc.tensor.matmul(
                            kps[:],
                            wkt_sb[:, ci, o * P : (o + 1) * P],
                            hn_blk[:, ci, :],
                            start=(ci == 0), stop=(ci == NCO - 1),
                        )
                    nc.scalar.add(k_sb[:, o, js:je], kps[:], bk_sb[:, o : o + 1])
                # vT = hn^T @ wv^T   ([j, c] layout)
                for jc in range(JBLK // P):
                    vps = psA.tile([P, IB], F32, tag="ps")
                    for ci in range(NCO):
                        nc.tensor.matmul(
                            vps[:],
                            hn_blk[:, ci, jc * P : (jc + 1) * P],
                            wvt_sb[:, ci, :],
                            start=(ci == 0), stop=(ci == NCO - 1),
                        )
                    jg = jb * (JBLK // P) + jc
                    nc.vector.tensor_add(vT_sb[:, jg, :], vps[:], bvb_sb[:])
                # q only for this core's half
                if jb < NJB // 2:
                    for o in range(NCO):
                        qps = psA.tile([P, IB], F32, tag="ps")
                        for ci in range(NCO):
                            nc.tensor.matmul(
                                qps[:],
                                wqt_sb[:, ci, o * P : (o + 1) * P],
                                hn_blk[:, ci, :],
                                start=(ci == 0), stop=(ci == NCO - 1),
                            )
                        nc.scalar.add(q_sb[:, o, js:je], qps[:], bq_sb[:, o : o + 1])

            # ---------------- phase 4: attention + proj + residual per i-block
            for ib in range(NIB):
                ibs, ibe = ib * IB, (ib + 1) * IB
                u_ps = [
                    psU.tile([P, IB], F32, tag="u", name=f"u_{ib}_{co}")
                    for co in range(NCO)
                ]
                d_ps = psD.tile([P, IB], F32, tag="d")

                def attnv(jg, ex):
                    for co in range(NCO):
                        nc.tensor.matmul(
                            u_ps[co],
                            vT_sb[:, jg, co * P : (co + 1) * P],
                            ex[:],
                            start=(jg == 0), stop=(jg == NJC - 1),
                        )
                    nc.tensor.matmul(
                        d_ps[:], ones_bf[:], ex[:],
                        start=(jg == 0), stop=(jg == NJC - 1),
                    )

                prev = None
                for jg in range(NJC):
                    sps = psA.tile([P, IB], F32, tag="ps")
                    for ci in range(NCO):
                        nc.tensor.matmul(
                            sps[:],
                            k_sb[:, ci, jg * P : (jg + 1) * P],
                            q_sb[:, ci, ibs:ibe],
                            start=(ci == 0), stop=(ci == NCO - 1),
                        )
                    ex = expp.tile([P, IB], BF16, tag="ex")
                    nc.scalar.activation(
                        ex[:], sps[:], mybir.ActivationFunctionType.Exp,
                        bias=0.0, scale=SCALE,
                    )
                    if prev is not None:
                        attnv(*prev)
                    prev = (jg, ex)
                attnv(*prev)

                u_sb = usb.tile([P, NCO, IB], BF16, tag="u_sb")
                for co in range(NCO):
                    nc.vector.tensor_copy(u_sb[:, co, :], u_ps[co])
                drec = drp.tile([P, IB], F32, tag="dr")
                nc.vector.reciprocal(drec[:], d_ps[:])
                x_blk = blk.tile([P, NCO, JBLK], F32, tag="xblk")
                nc.sync.dma_start(x_blk[:], x_r[:, :, ibs:ibe])
                for co in range(NCO):
                    nc.scalar.add(x_blk[:, co, :], x_blk[:, co, :], bp_sb[:, co : co + 1])
                out_sb = osb.tile([P, NCO, IB], F32, tag="out_sb")
                for o in range(NCO):
                    pps = psA.tile([P, IB], F32, tag="ps")
                    for ci in range(NCO):
                        nc.tensor.matmul(
                            pps[:],
                            wpt_sb[:, ci, o * P : (o + 1) * P],
                            u_sb[:, ci, :],
                            start=(ci == 0), stop=(ci == NCO - 1),
                        )
                    nc.vector.tensor_mul(out_sb[:, o, :], pps[:], drec[:])
                    nc.vector.tensor_add(out_sb[:, o, :], out_sb[:, o, :], x_blk[:, o, :])
                nc.sync.dma_start(out_r[:, :, ibs:ibe], out_sb[:])

    _split_multi_waits(nc)
    return nc


_NC_CACHE = []


def _get_nc():
    if not _NC_CACHE:
        _NC_CACHE.append(build_bass())
    return _NC_CACHE[0]


def _chunk_pc(v):
    """[512] per-channel vector -> [128, 4] (partition, chunk) layout."""
    return np.ascontiguousarray(v.reshape(NCO, P).T.astype(np.float32))


def kernel(x, gn_scale, gn_bias, wq, bq, wk, bk, wv, bv, wproj, bproj):
    x = np.asarray(x, dtype=np.float32)
    nc = _get_nc()

    aggm = np.zeros((P, 8), np.float32)
    for gg in range(8):
        aggm[gg * 16 : (gg + 1) * 16, gg] = 1.0 / 16.0
    bcm = np.zeros((8, P), np.float32)
    for gg in range(8):
        bcm[gg, gg * 16 : (gg + 1) * 16] = 1.0
    common = {
        "wqt": np.ascontiguousarray(np.asarray(wq, np.float32).T).astype(ml_dtypes.bfloat16),
        "wkt": np.ascontiguousarray(np.asarray(wk, np.float32).T).astype(ml_dtypes.bfloat16),
        "wvt": np.ascontiguousarray(np.asarray(wv, np.float32).T).astype(ml_dtypes.bfloat16),
        "wpt": np.ascontiguousarray(np.asarray(wproj, np.float32).T).astype(ml_dtypes.bfloat16),
        "bq": _chunk_pc(np.asarray(bq)),
        "bk": _chunk_pc(np.asarray(bk)),
        "bp": _chunk_pc(np.asarray(bproj)),
        "bvb": np.ascontiguousarray(np.tile(np.asarray(bv, np.float32)[None, :], (P, 1))),
        "gns": _chunk_pc(np.asarray(gn_scale)),
        "gnb": _chunk_pc(np.asarray(gn_bias)),
        "aggm": aggm,
        "bcm": bcm,
    }
    in_maps = []
    for r in range(8):
        s, h = r // 2, r % 2
        xs = x[s].reshape(C, HW)
        x_rot = np.ascontiguousarray(np.roll(xs, -h * IHALF, axis=1))
        in_maps.append({"x": x_rot, **common})

    res = run_bass_kernel_spmd(nc, in_maps, core_ids=list(range(8)))

    out = np.empty((B, C, HW), np.float32)
    for r in range(8):
        s, h = r // 2, r % 2
        out[s][:, h * IHALF : (h + 1) * IHALF] = res.results[r]["out"]
    return out.reshape(B, C, H, W)


# revision 21
# speedup vs baseline: 1.0634x; 1.0634x over previous
"""AttnBlock (GroupNorm + single-head spatial attention + proj + residual)
on 8 Trainium2 NeuronCores via Bass/Tile.

Sharding: batch b=4 -> 4 samples x 2 cores each. Each core receives its
sample's x with its query-half columns rotated to the front (attention is
permutation-invariant over key positions), computes GroupNorm + k + v for
the full sample (redundant with its pair core) and q/attention/proj for its
2048 query positions. No cross-core communication.
"""

import numpy as np
import ml_dtypes

import concourse.bass as bass
import concourse.tile as tile
import concourse.mybir as mybir
from concourse.bass_utils import run_bass_kernel_spmd
from concourse.vector_clock import ScopedClock, VectorClock
from concourse.tile_scheduler import N_PROCS

# ---------------------------------------------------------------- constants
B, C, H, W = 4, 512, 64, 64
HW = H * W            # 4096
P = 128
NCO = C // P          # 4 channel chunks of 128
G = 32                # groups
IHALF = HW // 2       # 2048 query columns per core
IB = 512              # i-block width
NIB = IHALF // IB     # 4
JBLK = 512            # column block for GN/qkv phases
NJB = HW // JBLK      # 8
NJC = HW // P         # 32 j-chunks of 128
EPS = 1e-6
SCALE = float(1.0 / np.sqrt(C))
F32 = mybir.dt.float32
BF16 = mybir.dt.bfloat16


# ------------------------------------------------- walrus single-wait fixes
class _TileContextFix(tile.TileContext):
    """TileContext whose tail drain splits sem waits across NOPs.

    The walrus build here rejects instructions carrying more than one sync
    wait ("Too many sync wait commands"), so the stock tail drain (one wait
    per outstanding proc) cannot codegen. Emit one single-wait NOP per proc
    before a wait-free drain.
    """

    def _drain_and_barrier(self, tick_clock, wait_clock):
        gc = tick_clock.global_clock
        for p in range(N_PROCS):
            if gc[p] == 0:
                continue
            partial = VectorClock([gc[q] if q == p else 0 for q in range(N_PROCS)])
            nop_inst = self.nc.sync.nop(nofuse=True, hint=f"tail_wait_{p}")
            wait_clock.add_sem_waits(nop_inst.ins, ScopedClock({None: partial}))
        self.nc.sync.drain()
        self.nc.all_engine_barrier()
        assert self.sems is not None
        popped = self.nc._tile_sem_poison_stack.pop()
        assert popped is self._sem_poison
        self.nc.clear_and_free_semaphores(list(self.sems.allocated().values()))
        self.nc.all_engine_barrier()


def _split_multi_waits(nc):
    """Split any instruction with N>1 sync waits into N-1 single-wait NOPs
    prepended on the same engine (same stream -> same ordering; sems are
    monotonic so waiting earlier is safe)."""
    fn = nc.m.functions[0]
    n_split = 0
    for bb in fn.blocks:
        insts = list(bb.instructions)
        out = []
        for inst in insts:
            si = inst.sync_info
            if si is not None and si.on_wait and len(si.on_wait) > 1:
                waits = list(si.on_wait)
                for w in waits[:-1]:
                    nop = mybir.InstNoOp(
                        name=nc.get_next_instruction_name(),
                        engine=inst.engine,
                        sync_info=mybir.SyncInfo(on_wait=[w], on_update=[]),
                        bass_nofuse=True,
                        ins=[],
                        outs=[],
                    )
                    out.append(nop)
                    n_split += 1
                inst.sync_info = mybir.SyncInfo(
                    on_wait=[waits[-1]], on_update=list(si.on_update or [])
                )
            out.append(inst)
        if len(out) != len(insts):
            bb.instructions[:] = out
    return n_split


# ------------------------------------------------------------- the kernel
def build_bass():
    nc = bass.Bass("TRN2", target_bir_lowering=False, debug=False, num_devices=8)

    x_d = nc.dram_tensor("x", [C, HW], F32, kind="ExternalInput")
    xh_d = nc.dram_tensor("xh", [C, HW], BF16, kind="ExternalInput")
    wqt_d = nc.dram_tensor("wqt", [C, C], BF16, kind="ExternalInput")
    wkt_d = nc.dram_tensor("wkt", [C, C], BF16, kind="ExternalInput")
    wvt_d = nc.dram_tensor("wvt", [C, C], BF16, kind="ExternalInput")
    wpt_d = nc.dram_tensor("wpt", [C, C], BF16, kind="ExternalInput")
    bq_d = nc.dram_tensor("bq", [P, NCO], F32, kind="ExternalInput")
    bk_d = nc.dram_tensor("bk", [P, NCO], F32, kind="ExternalInput")
    bp_d = nc.dram_tensor("bp", [P, NCO], F32, kind="ExternalInput")
    bvb_d = nc.dram_tensor("bvb", [P, C], F32, kind="ExternalInput")
    gns_d = nc.dram_tensor("gns", [P, NCO], F32, kind="ExternalInput")
    gnb_d = nc.dram_tensor("gnb", [P, NCO], F32, kind="ExternalInput")
    aggm_d = nc.dram_tensor("aggm", [P, 8], F32, kind="ExternalInput")
    bcm_d = nc.dram_tensor("bcm", [8, P], F32, kind="ExternalInput")
    out_d = nc.dram_tensor("out", [C, IHALF], F32, kind="ExternalOutput")

    x_r = x_d.ap().rearrange("(co p) j -> p co j", p=P)        # [128,4,4096]
    xh_r = xh_d.ap().rearrange("(co p) j -> p co j", p=P)
    out_r = out_d.ap().rearrange("(co p) i -> p co i", p=P)    # [128,4,2048]

    with _TileContextFix(nc) as tc:
        with (
            tc.tile_pool(name="consts", bufs=1) as consts,
            tc.tile_pool(name="xbf", bufs=1) as xbf,
            tc.tile_pool(name="blk", bufs=3) as blk,
            tc.tile_pool(name="kqv", bufs=1) as kqv,
            tc.tile_pool(name="stat", bufs=1) as stat,
            tc.tile_pool(name="expp", bufs=6) as expp,
            tc.tile_pool(name="usb", bufs=2) as usb,
            tc.tile_pool(name="drp", bufs=2) as drp,
            tc.tile_pool(name="osb", bufs=2) as osb,
            tc.tile_pool(name="psA", bufs=3, space="PSUM") as psA,
            tc.tile_pool(name="psU", bufs=4, space="PSUM") as psU,
            tc.tile_pool(name="psD", bufs=1, space="PSUM") as psD,
        ):
            # ---------------- phase 1 loads first (off the weight queues)
            x_bf = xbf.tile([P, NCO, HW], BF16)
            for jb in (6, 7, 0, 1, 2, 3, 4, 5):
                js, je = jb * JBLK, (jb + 1) * JBLK
                eng = nc.gpsimd if jb >= 6 else nc.sync
                eng.dma_start(x_bf[:, :, js:je], xh_r[:, :, js:je])

            # ---------------- constants
            wqt_sb = consts.tile([P, NCO, C], BF16)
            nc.scalar.dma_start(wqt_sb[:], wqt_d.ap().rearrange("(ci p) o -> p ci o", p=P))
            wkt_sb = consts.tile([P, NCO, C], BF16)
            nc.scalar.dma_start(wkt_sb[:], wkt_d.ap().rearrange("(ci p) o -> p ci o", p=P))
            wvt_sb = consts.tile([P, NCO, C], BF16)
            nc.scalar.dma_start(wvt_sb[:], wvt_d.ap().rearrange("(ci p) o -> p ci o", p=P))
            wpt_sb = consts.tile([P, NCO, C], BF16)
            nc.scalar.dma_start(wpt_sb[:], wpt_d.ap().rearrange("(ci p) o -> p ci o", p=P))
            bq_sb = consts.tile([P, NCO], F32)
            nc.scalar.dma_start(bq_sb[:], bq_d.ap())
            bk_sb = consts.tile([P, NCO], F32)
            nc.scalar.dma_start(bk_sb[:], bk_d.ap())
            bp_sb = consts.tile([P, NCO], F32)
            nc.scalar.dma_start(bp_sb[:], bp_d.ap())
            bvb_sb = consts.tile([P, C], F32)
            nc.scalar.dma_start(bvb_sb[:], bvb_d.ap())
            gns_sb = consts.tile([P, NCO], F32)
            nc.scalar.dma_start(gns_sb[:], gns_d.ap())
            gnb_sb = consts.tile([P, NCO], F32)
            nc.scalar.dma_start(gnb_sb[:], gnb_d.ap())
            aggm_sb = consts.tile([P, 8], F32)
            nc.scalar.dma_start(aggm_sb[:], aggm_d.ap())
            bcm_sb = consts.tile([8, P], F32)
            nc.scalar.dma_start(bcm_sb[:], bcm_d.ap())
            ones_bf = consts.tile([P, P], BF16)
            nc.vector.memset(ones_bf[:], 1.0)
            eps_sb = consts.tile([8, 1], F32)
            nc.vector.memset(eps_sb[:], EPS)

            DVE_BLKS = [0, 1, 2, 3, 4, 5]
            ACT_BLKS = [6, 7]
            stats = stat.tile([P, NCO, len(DVE_BLKS), 6], F32)
            asum = stat.tile([P, NCO, 2, 2], F32)
            mv = stat.tile([P, NCO, 2], F32)

            # ---------------- phase 1: per-channel stats (DVE + ACT split)
            for bi, jb in enumerate(DVE_BLKS):
                js, je = jb * JBLK, (jb + 1) * JBLK
                for co in range(NCO):
                    nc.vector.bn_stats(stats[:, co, bi, :], x_bf[:, co, js:je])
            scr = stat.tile([P, JBLK], BF16)
            for bi, jb in enumerate(ACT_BLKS):
                js, je = jb * JBLK, (jb + 1) * JBLK
                for co in range(NCO):
                    nc.scalar.activation(
                        scr[:], x_bf[:, co, js:je],
                        mybir.ActivationFunctionType.Identity,
                        accum_out=asum[:, co, bi, 0:1],
                    )
                    nc.scalar.activation(
                        scr[:], x_bf[:, co, js:je],
                        mybir.ActivationFunctionType.Square,
                        accum_out=asum[:, co, bi, 1:2],
                    )

            # ---------------- phase 3: group stats -> per-channel affine A, B
            for co in range(NCO):
                nc.vector.bn_aggr(mv[:, co, :], stats[:, co, :, :])
            m2 = stat.tile([P, NCO], F32)
            nc.vector.tensor_mul(m2[:], mv[:, :, 0], mv[:, :, 0])
            nc.vector.tensor_add(mv[:, :, 1], mv[:, :, 1], m2[:])  # E[x^2] (DVE blocks)
            # merge ACT-block sums: stat = (stat6 * 3072 + act_sum) / 4096
            n_dve = float(len(DVE_BLKS) * JBLK)
            sum_t = stat.tile([P, NCO], F32)
            nc.vector.tensor_add(sum_t[:], asum[:, :, 0, 0], asum[:, :, 1, 0])
            ssq_t = stat.tile([P, NCO], F32)
            nc.vector.tensor_add(ssq_t[:], asum[:, :, 0, 1], asum[:, :, 1, 1])
            nc.vector.tensor_scalar(
                mv[:, :, 0], mv[:, :, 0], n_dve, None, op0=mybir.AluOpType.mult
            )
            nc.vector.tensor_add(mv[:, :, 0], mv[:, :, 0], sum_t[:])
            nc.vector.tensor_scalar(
                mv[:, :, 0], mv[:, :, 0], 1.0 / HW, None, op0=mybir.AluOpType.mult
            )
            nc.vector.tensor_scalar(
                mv[:, :, 1], mv[:, :, 1], n_dve, None, op0=mybir.AluOpType.mult
            )
            nc.vector.tensor_add(mv[:, :, 1], mv[:, :, 1], ssq_t[:])
            nc.vector.tensor_scalar(
                mv[:, :, 1], mv[:, :, 1], 1.0 / HW, None, op0=mybir.AluOpType.mult
            )
            ps_s = psA.tile([P, IB], F32, tag="ps")
            nc.tensor.matmul(
                ps_s[:8, : NCO * 2],
                aggm_sb[:],
                mv[:].rearrange("p co s -> p (co s)"),
                start=True, stop=True,
            )
            grp = stat.tile([8, NCO, 2], F32)
            nc.vector.tensor_copy(grp[:], ps_s[:8, : NCO * 2])
            g2 = stat.tile([8, NCO], F32)
            nc.vector.tensor_mul(g2[:], grp[:, :, 0], grp[:, :, 0])
            nc.vector.tensor_tensor(
                grp[:, :, 1], grp[:, :, 1], g2[:], mybir.AluOpType.subtract
            )  # var_g
            nc.scalar.activation(
                grp[:, :, 1], grp[:, :, 1], mybir.ActivationFunctionType.Sqrt,
                bias=eps_sb[:], scale=1.0,
            )
            nc.vector.reciprocal(grp[:, :, 1], grp[:, :, 1])  # rstd_g
            ps_b = psA.tile([P, IB], F32, tag="ps")
            nc.tensor.matmul(
                ps_b[:, : NCO * 2],
                bcm_sb[:],
                grp[:].rearrange("g co s -> g (co s)"),
                start=True, stop=True,
            )
            mvb = stat.tile([P, NCO, 2], F32)  # per-channel (mean_g, rstd_g)
            nc.vector.tensor_copy(mvb[:], ps_b[:, : NCO * 2])
            A = stat.tile([P, NCO], F32)
            nc.vector.tensor_mul(A[:], mvb[:, :, 1], gns_sb[:])
            t2 = stat.tile([P, NCO], F32)
            nc.vector.tensor_mul(t2[:], mvb[:, :, 0], A[:])
            Bc = stat.tile([P, NCO], F32)
            nc.vector.tensor_tensor(Bc[:], gnb_sb[:], t2[:], mybir.AluOpType.subtract)

            # ---------------- phase 2 prep: fold GN affine into weights
            # q/k/v = w @ (A*x + B) + b = (w.A) @ x + (w @ B + b); the
            # B-terms are per-output-channel constants computed with tiny
            # N=1 matmuls, then the big matmuls read x_bf directly.
            Bc_bf = stat.tile([P, NCO], BF16)
            nc.vector.tensor_copy(Bc_bf[:], Bc[:])
            kbias = stat.tile([P, NCO], F32)
            qbias = stat.tile([P, NCO], F32)
            for w_sb, b_sb, bias_col in (
                (wkt_sb, bk_sb, kbias),
                (wqt_sb, bq_sb, qbias),
            ):
                for o in range(NCO):
                    tps = psA.tile([P, IB], F32, tag="ps", name=f"tps_{o}")
                    for ci in range(NCO):
                        nc.tensor.matmul(
                            tps[:, 0:1],
                            w_sb[:, ci, o * P : (o + 1) * P],
                            Bc_bf[:, ci : ci + 1],
                            start=(ci == 0), stop=(ci == NCO - 1),
                        )
                    nc.vector.tensor_add(
                        bias_col[:, o : o + 1], tps[:, 0:1], b_sb[:, o : o + 1]
                    )
            # r[c] = B @ wvT, broadcast over partitions, + bv broadcast
            rps = psA.tile([P, IB], F32, tag="ps")
            for ci in range(NCO):
                nc.tensor.matmul(
                    rps[:1, :],
                    Bc_bf[:, ci : ci + 1],
                    wvt_sb[:, ci, :],
                    start=(ci == 0), stop=(ci == NCO - 1),
                )
            r_bf = stat.tile([1, C], BF16)
            nc.vector.tensor_copy(r_bf[:], rps[:1, :])
            vbps = psA.tile([P, IB], F32, tag="ps")
            nc.tensor.matmul(
                vbps[:, :], ones_bf[0:1, :], r_bf[:], start=True, stop=True
            )
            vbias = stat.tile([P, C], F32)
            nc.vector.tensor_add(vbias[:], vbps[:], bvb_sb[:])
            def scale_w(w_sb, name):
                # w' = w * A (per input channel = per partition), new tile so
                # the unscaled-weight bias matmuls don't serialize against it
                w_s = kqv.tile([P, NCO, C], BF16, name=name)
                for ci in range(NCO):
                    nc.vector.tensor_scalar_mul(
                        w_s[:, ci, :], w_sb[:, ci, :], A[:, ci : ci + 1]
                    )
                return w_s

            # ---------------- phase 2: k, vT, q straight from x_bf
            k_sb = kqv.tile([P, NCO, HW], BF16)
            q_sb = kqv.tile([P, NCO, IHALF], BF16)
            vT_sb = kqv.tile([P, NJC, C], BF16)
            wkt_s = scale_w(wkt_sb, "wkt_s")
            for jb in range(NJB):
                js, je = jb * JBLK, (jb + 1) * JBLK
                for o in range(NCO):
                    kps = psA.tile([P, IB], F32, tag="ps")
                    for ci in range(NCO):
                        nc.tensor.matmul(
                            kps[:],
                            wkt_s[:, ci, o * P : (o + 1) * P],
                            x_bf[:, ci, js:je],
                            start=(ci == 0), stop=(ci == NCO - 1),
                        )
                    nc.scalar.add(k_sb[:, o, js:je], kps[:], kbias[:, o : o + 1])
            wvt_s = scale_w(wvt_sb, "wvt_s")
            for jb in range(NJB):
                js, je = jb * JBLK, (jb + 1) * JBLK
                for jc in range(JBLK // P):
                    vps = psA.tile([P, IB], F32, tag="ps")
                    for ci in range(NCO):
                        nc.tensor.matmul(
                            vps[:],
                            x_bf[:, ci, js + jc * P : js + (jc + 1) * P],
                            wvt_s[:, ci, :],
                            start=(ci == 0), stop=(ci == NCO - 1),
                        )
                    jg = jb * (JBLK // P) + jc
                    nc.vector.tensor_add(vT_sb[:, jg, :], vps[:], vbias[:])
            wqt_s = scale_w(wqt_sb, "wqt_s")
            for jb in range(NJB // 2):
                js, je = jb * JBLK, (jb + 1) * JBLK
                for o in range(NCO):
                    qps = psA.tile([P, IB], F32, tag="ps")
                    for ci in range(NCO):
                        nc.tensor.matmul(
                            qps[:],
                            wqt_s[:, ci, o * P : (o + 1) * P],
                            x_bf[:, ci, js:je],
                            start=(ci == 0), stop=(ci == NCO - 1),
                        )
                    nc.scalar.add(q_sb[:, o, js:je], qps[:], qbias[:, o : o + 1])

            # ---------------- phase 4: attention + proj + residual per i-block
            pending = []
            for ib in range(NIB):
                ibs, ibe = ib * IB, (ib + 1) * IB
                u_ps = [
                    psU.tile([P, IB], F32, tag="u", name=f"u_{ib}_{co}")
                    for co in range(NCO)
                ]
                d_ps = psD.tile([P, IB], F32, tag="d")

                def attnv(jg, ex):
                    for co in range(NCO):
                        nc.tensor.matmul(
                            u_ps[co],
                            vT_sb[:, jg, co * P : (co + 1) * P],
                            ex[:],
                            start=(jg == 0), stop=(jg == NJC - 1),
                        )
                    nc.tensor.matmul(
                        d_ps[:], ones_bf[:], ex[:],
                        start=(jg == 0), stop=(jg == NJC - 1),
                    )

                prev = None
                for jg in range(NJC):
                    sps = psA.tile([P, IB], F32, tag="ps")
                    for ci in range(NCO):
                        nc.tensor.matmul(
                            sps[:],
                            k_sb[:, ci, jg * P : (jg + 1) * P],
                            q_sb[:, ci, ibs:ibe],
                            start=(ci == 0), stop=(ci == NCO - 1),
                        )
                    ex = expp.tile([P, IB], BF16, tag="ex")
                    nc.scalar.activation(
                        ex[:], sps[:], mybir.ActivationFunctionType.Exp,
                        bias=0.0, scale=SCALE,
                    )
                    if prev is not None:
                        attnv(*prev)
                    prev = (jg, ex)
                attnv(*prev)

                u_sb = usb.tile([P, NCO, IB], BF16, tag="u_sb")
                for co in range(NCO):
                    nc.vector.tensor_copy(u_sb[:, co, :], u_ps[co])
                drec = drp.tile([P, IB], F32, tag="dr")
                nc.vector.reciprocal(drec[:], d_ps[:])
                x_blk = blk.tile([P, NCO, JBLK], F32, tag="xblk")
                nc.sync.dma_start(x_blk[:], x_r[:, :, ibs:ibe])
                for co in range(NCO):
                    nc.scalar.add(x_blk[:, co, :], x_blk[:, co, :], bp_sb[:, co : co + 1])

                def proj_epilogue(ibs=ibs, ibe=ibe, u_sb=u_sb, drec=drec, x_blk=x_blk):
                    out_sb = osb.tile([P, NCO, IB], F32, tag="out_sb")
                    for o in range(NCO):
                        pps = psA.tile([P, IB], F32, tag="ps", name=f"pps_{ibs}_{o}")
                        for ci in range(NCO):
                            nc.tensor.matmul(
                                pps[:],
                                wpt_sb[:, ci, o * P : (o + 1) * P],
                                u_sb[:, ci, :],
                                start=(ci == 0), stop=(ci == NCO - 1),
                            )
                        nc.vector.tensor_mul(out_sb[:, o, :], pps[:], drec[:])
                        nc.vector.tensor_add(
                            out_sb[:, o, :], out_sb[:, o, :], x_blk[:, o, :]
                        )
                        nc.sync.dma_start(out_r[:, o, ibs:ibe], out_sb[:, o, :])

                # defer this block's proj+epilogue until the next block's
                # attention loop is emitted so PE has ready work at the seam
                pending.append(proj_epilogue)
                if len(pending) > 1:
                    pending.pop(0)()
            for fn in pending:
                fn()

    _split_multi_waits(nc)
    return nc


_NC_CACHE = []


def _get_nc():
    if not _NC_CACHE:
        _NC_CACHE.append(build_bass())
    return _NC_CACHE[0]


def _chunk_pc(v):
    """[512] per-channel vector -> [128, 4] (partition, chunk) layout."""
    return np.ascontiguousarray(v.reshape(NCO, P).T.astype(np.float32))


def kernel(x, gn_scale, gn_bias, wq, bq, wk, bk, wv, bv, wproj, bproj):
    x = np.asarray(x, dtype=np.float32)
    nc = _get_nc()

    aggm = np.zeros((P, 8), np.float32)
    for gg in range(8):
        aggm[gg * 16 : (gg + 1) * 16, gg] = 1.0 / 16.0
    bcm = np.zeros((8, P), np.float32)
    for gg in range(8):
        bcm[gg, gg * 16 : (gg + 1) * 16] = 1.0
    common = {
        "wqt": np.ascontiguousarray(np.asarray(wq, np.float32).T).astype(ml_dtypes.bfloat16),
        "wkt": np.ascontiguousarray(np.asarray(wk, np.float32).T).astype(ml_dtypes.bfloat16),
        "wvt": np.ascontiguousarray(np.asarray(wv, np.float32).T).astype(ml_dtypes.bfloat16),
        "wpt": np.ascontiguousarray(np.asarray(wproj, np.float32).T).astype(ml_dtypes.bfloat16),
        "bq": _chunk_pc(np.asarray(bq)),
        "bk": _chunk_pc(np.asarray(bk)),
        "bp": _chunk_pc(np.asarray(bproj)),
        "bvb": np.ascontiguousarray(np.tile(np.asarray(bv, np.float32)[None, :], (P, 1))),
        "gns": _chunk_pc(np.asarray(gn_scale)),
        "gnb": _chunk_pc(np.asarray(gn_bias)),
        "aggm": aggm,
        "bcm": bcm,
    }
    in_maps = []
    for r in range(8):
        s, h = r // 2, r % 2
        xs = x[s].reshape(C, HW)
        x_rot = np.ascontiguousarray(np.roll(xs, -h * IHALF, axis=1))
        in_maps.append({
            "x": x_rot,
            "xh": x_rot.astype(ml_dtypes.bfloat16),
            **common,
        })

    res = run_bass_kernel_spmd(nc, in_maps, core_ids=list(range(8)))

    out = np.empty((B, C, HW), np.float32)
    for r in range(8):
        s, h = r // 2, r % 2
        out[s][:, h * IHALF : (h + 1) * IHALF] = res.results[r]["out"]
    return out.reshape(B, C, H, W)


# revision 26
# speedup vs baseline: 1.0641x; 1.0007x over previous
"""AttnBlock (GroupNorm + single-head spatial attention + proj + residual)
on 8 Trainium2 NeuronCores via Bass/Tile.

Sharding: batch b=4 -> 4 samples x 2 cores each. Each core receives its
sample's x with its query-half columns rotated to the front (attention is
permutation-invariant over key positions), computes GroupNorm + k + v for
the full sample (redundant with its pair core) and q/attention/proj for its
2048 query positions. No cross-core communication.
"""

import numpy as np
import ml_dtypes

import concourse.bass as bass
import concourse.tile as tile
import concourse.mybir as mybir
from concourse.bass_utils import run_bass_kernel_spmd
from concourse.vector_clock import ScopedClock, VectorClock
from concourse.tile_scheduler import N_PROCS

# ---------------------------------------------------------------- constants
B, C, H, W = 4, 512, 64, 64
HW = H * W            # 4096
P = 128
NCO = C // P          # 4 channel chunks of 128
G = 32                # groups
IHALF = HW // 2       # 2048 query columns per core
IB = 512              # i-block width
NIB = IHALF // IB     # 4
JBLK = 512            # column block for GN/qkv phases
NJB = HW // JBLK      # 8
NJC = HW // P         # 32 j-chunks of 128
EPS = 1e-6
SCALE = float(1.0 / np.sqrt(C))
F32 = mybir.dt.float32
BF16 = mybir.dt.bfloat16


# ------------------------------------------------- walrus single-wait fixes
class _TileContextFix(tile.TileContext):
    """TileContext whose tail drain splits sem waits across NOPs.

    The walrus build here rejects instructions carrying more than one sync
    wait ("Too many sync wait commands"), so the stock tail drain (one wait
    per outstanding proc) cannot codegen. Emit one single-wait NOP per proc
    before a wait-free drain.
    """

    def _drain_and_barrier(self, tick_clock, wait_clock):
        gc = tick_clock.global_clock
        for p in range(N_PROCS):
            if gc[p] == 0:
                continue
            partial = VectorClock([gc[q] if q == p else 0 for q in range(N_PROCS)])
            nop_inst = self.nc.sync.nop(nofuse=True, hint=f"tail_wait_{p}")
            wait_clock.add_sem_waits(nop_inst.ins, ScopedClock({None: partial}))
        self.nc.sync.drain()
        self.nc.all_engine_barrier()
        assert self.sems is not None
        popped = self.nc._tile_sem_poison_stack.pop()
        assert popped is self._sem_poison
        self.nc.clear_and_free_semaphores(list(self.sems.allocated().values()))


def _split_multi_waits(nc):
    """Split any instruction with N>1 sync waits into N-1 single-wait NOPs
    prepended on the same engine (same stream -> same ordering; sems are
    monotonic so waiting earlier is safe)."""
    fn = nc.m.functions[0]
    n_split = 0
    for bb in fn.blocks:
        insts = list(bb.instructions)
        out = []
        for inst in insts:
            si = inst.sync_info
            if si is not None and si.on_wait and len(si.on_wait) > 1:
                waits = list(si.on_wait)
                for w in waits[:-1]:
                    nop = mybir.InstNoOp(
                        name=nc.get_next_instruction_name(),
                        engine=inst.engine,
                        sync_info=mybir.SyncInfo(on_wait=[w], on_update=[]),
                        bass_nofuse=True,
                        ins=[],
                        outs=[],
                    )
                    out.append(nop)
                    n_split += 1
                inst.sync_info = mybir.SyncInfo(
                    on_wait=[waits[-1]], on_update=list(si.on_update or [])
                )
            out.append(inst)
        if len(out) != len(insts):
            bb.instructions[:] = out
    return n_split


# ------------------------------------------------------------- the kernel
def build_bass():
    nc = bass.Bass("TRN2", target_bir_lowering=False, debug=False, num_devices=8)

    x_d = nc.dram_tensor("x", [C, HW], F32, kind="ExternalInput")
    xh_d = nc.dram_tensor("xh", [C, HW], BF16, kind="ExternalInput")
    wqt_d = nc.dram_tensor("wqt", [C, C], BF16, kind="ExternalInput")
    wkt_d = nc.dram_tensor("wkt", [C, C], BF16, kind="ExternalInput")
    wvt_d = nc.dram_tensor("wvt", [C, C], BF16, kind="ExternalInput")
    wpt_d = nc.dram_tensor("wpt", [C, C], BF16, kind="ExternalInput")
    bq_d = nc.dram_tensor("bq", [P, NCO], F32, kind="ExternalInput")
    bk_d = nc.dram_tensor("bk", [P, NCO], F32, kind="ExternalInput")
    bp_d = nc.dram_tensor("bp", [P, NCO], F32, kind="ExternalInput")
    bvb_d = nc.dram_tensor("bvb", [P, C], F32, kind="ExternalInput")
    gns_d = nc.dram_tensor("gns", [P, NCO], F32, kind="ExternalInput")
    gnb_d = nc.dram_tensor("gnb", [P, NCO], F32, kind="ExternalInput")
    aggm_d = nc.dram_tensor("aggm", [P, 8], F32, kind="ExternalInput")
    bcm_d = nc.dram_tensor("bcm", [8, P], F32, kind="ExternalInput")
    out_d = nc.dram_tensor("out", [C, IHALF], F32, kind="ExternalOutput")

    x_r = x_d.ap().rearrange("(co p) j -> p co j", p=P)        # [128,4,4096]
    xh_r = xh_d.ap().rearrange("(co p) j -> p co j", p=P)
    out_r = out_d.ap().rearrange("(co p) i -> p co i", p=P)    # [128,4,2048]

    with _TileContextFix(nc) as tc:
        with (
            tc.tile_pool(name="consts", bufs=1) as consts,
            tc.tile_pool(name="xbf", bufs=1) as xbf,
            tc.tile_pool(name="blk", bufs=3) as blk,
            tc.tile_pool(name="kqv", bufs=1) as kqv,
            tc.tile_pool(name="stat", bufs=1) as stat,
            tc.tile_pool(name="expp", bufs=6) as expp,
            tc.tile_pool(name="usb", bufs=2) as usb,
            tc.tile_pool(name="drp", bufs=2) as drp,
            tc.tile_pool(name="osb", bufs=2) as osb,
            tc.tile_pool(name="psA", bufs=3, space="PSUM") as psA,
            tc.tile_pool(name="psU", bufs=4, space="PSUM") as psU,
            tc.tile_pool(name="psD", bufs=1, space="PSUM") as psD,
        ):
            # ---------------- phase 1 loads first (off the weight queues)
            x_bf = xbf.tile([P, NCO, HW], BF16)
            for jb in (6, 7, 0, 1, 2, 3, 4, 5):
                js, je = jb * JBLK, (jb + 1) * JBLK
                eng = nc.gpsimd if jb >= 6 else nc.sync
                eng.dma_start(x_bf[:, :, js:je], xh_r[:, :, js:je])

            # ---------------- constants
            bq_sb = consts.tile([P, NCO], F32)
            nc.sync.dma_start(bq_sb[:], bq_d.ap())
            bk_sb = consts.tile([P, NCO], F32)
            nc.sync.dma_start(bk_sb[:], bk_d.ap())
            bp_sb = consts.tile([P, NCO], F32)
            nc.sync.dma_start(bp_sb[:], bp_d.ap())
            bvb_sb = consts.tile([P, C], F32)
            nc.sync.dma_start(bvb_sb[:], bvb_d.ap())
            gns_sb = consts.tile([P, NCO], F32)
            nc.sync.dma_start(gns_sb[:], gns_d.ap())
            gnb_sb = consts.tile([P, NCO], F32)
            nc.sync.dma_start(gnb_sb[:], gnb_d.ap())
            aggm_sb = consts.tile([P, 8], F32)
            nc.sync.dma_start(aggm_sb[:], aggm_d.ap())
            bcm_sb = consts.tile([8, P], F32)
            nc.sync.dma_start(bcm_sb[:], bcm_d.ap())
            wqt_sb = consts.tile([P, NCO, C], BF16)
            nc.sync.dma_start(wqt_sb[:], wqt_d.ap().rearrange("(ci p) o -> p ci o", p=P))
            wkt_sb = consts.tile([P, NCO, C], BF16)
            nc.sync.dma_start(wkt_sb[:], wkt_d.ap().rearrange("(ci p) o -> p ci o", p=P))
            wvt_sb = consts.tile([P, NCO, C], BF16)
            nc.sync.dma_start(wvt_sb[:], wvt_d.ap().rearrange("(ci p) o -> p ci o", p=P))
            wpt_sb = consts.tile([P, NCO, C], BF16)
            nc.sync.dma_start(wpt_sb[:], wpt_d.ap().rearrange("(ci p) o -> p ci o", p=P))
            ones_bf = consts.tile([P, P], BF16)
            nc.vector.memset(ones_bf[:], 1.0)
            eps_sb = consts.tile([8, 1], F32)
            nc.vector.memset(eps_sb[:], EPS)

            DVE_BLKS = [0, 1, 2, 3, 4, 5]
            ACT_BLKS = [6, 7]
            stats = stat.tile([P, NCO, len(DVE_BLKS), 6], F32)
            asum = stat.tile([P, NCO, 2, 2], F32)
            mv = stat.tile([P, NCO, 2], F32)

            # ---------------- phase 1: per-channel stats (DVE + ACT split)
            for bi, jb in enumerate(DVE_BLKS):
                js, je = jb * JBLK, (jb + 1) * JBLK
                for co in range(NCO):
                    nc.vector.bn_stats(stats[:, co, bi, :], x_bf[:, co, js:je])
            scr = stat.tile([P, JBLK], BF16)
            for bi, jb in enumerate(ACT_BLKS):
                js, je = jb * JBLK, (jb + 1) * JBLK
                for co in range(NCO):
                    nc.scalar.activation(
                        scr[:], x_bf[:, co, js:je],
                        mybir.ActivationFunctionType.Identity,
                        accum_out=asum[:, co, bi, 0:1],
                    )
                    nc.scalar.activation(
                        scr[:], x_bf[:, co, js:je],
                        mybir.ActivationFunctionType.Square,
                        accum_out=asum[:, co, bi, 1:2],
                    )

            # ---------------- phase 3: group stats -> per-channel affine A, B
            for co in range(NCO):
                nc.vector.bn_aggr(mv[:, co, :], stats[:, co, :, :])
            m2 = stat.tile([P, NCO], F32)
            nc.vector.tensor_mul(m2[:], mv[:, :, 0], mv[:, :, 0])
            nc.vector.tensor_add(mv[:, :, 1], mv[:, :, 1], m2[:])  # E[x^2] (DVE blocks)
            # merge ACT-block sums: stat = (stat6 * 3072 + act_sum) / 4096
            n_dve = float(len(DVE_BLKS) * JBLK)
            sum_t = stat.tile([P, NCO], F32)
            nc.vector.tensor_add(sum_t[:], asum[:, :, 0, 0], asum[:, :, 1, 0])
            ssq_t = stat.tile([P, NCO], F32)
            nc.vector.tensor_add(ssq_t[:], asum[:, :, 0, 1], asum[:, :, 1, 1])
            nc.vector.tensor_scalar(
                mv[:, :, 0], mv[:, :, 0], n_dve, None, op0=mybir.AluOpType.mult
            )
            nc.vector.tensor_add(mv[:, :, 0], mv[:, :, 0], sum_t[:])
            nc.vector.tensor_scalar(
                mv[:, :, 0], mv[:, :, 0], 1.0 / HW, None, op0=mybir.AluOpType.mult
            )
            nc.vector.tensor_scalar(
                mv[:, :, 1], mv[:, :, 1], n_dve, None, op0=mybir.AluOpType.mult
            )
            nc.vector.tensor_add(mv[:, :, 1], mv[:, :, 1], ssq_t[:])
            nc.vector.tensor_scalar(
                mv[:, :, 1], mv[:, :, 1], 1.0 / HW, None, op0=mybir.AluOpType.mult
            )
            ps_s = psA.tile([P, IB], F32, tag="ps")
            nc.tensor.matmul(
                ps_s[:8, : NCO * 2],
                aggm_sb[:],
                mv[:].rearrange("p co s -> p (co s)"),
                start=True, stop=True,
            )
            grp = stat.tile([8, NCO, 2], F32)
            nc.vector.tensor_copy(grp[:], ps_s[:8, : NCO * 2])
            g2 = stat.tile([8, NCO], F32)
            nc.vector.tensor_mul(g2[:], grp[:, :, 0], grp[:, :, 0])
            nc.vector.tensor_tensor(
                grp[:, :, 1], grp[:, :, 1], g2[:], mybir.AluOpType.subtract
            )  # var_g
            nc.scalar.activation(
                grp[:, :, 1], grp[:, :, 1], mybir.ActivationFunctionType.Sqrt,
                bias=eps_sb[:], scale=1.0,
            )
            nc.vector.reciprocal(grp[:, :, 1], grp[:, :, 1])  # rstd_g
            ps_b = psA.tile([P, IB], F32, tag="ps")
            nc.tensor.matmul(
                ps_b[:, : NCO * 2],
                bcm_sb[:],
                grp[:].rearrange("g co s -> g (co s)"),
                start=True, stop=True,
            )
            mvb = stat.tile([P, NCO, 2], F32)  # per-channel (mean_g, rstd_g)
            nc.vector.tensor_copy(mvb[:], ps_b[:, : NCO * 2])
            A = stat.tile([P, NCO], F32)
            nc.vector.tensor_mul(A[:], mvb[:, :, 1], gns_sb[:])
            t2 = stat.tile([P, NCO], F32)
            nc.vector.tensor_mul(t2[:], mvb[:, :, 0], A[:])
            Bc = stat.tile([P, NCO], F32)
            nc.vector.tensor_tensor(Bc[:], gnb_sb[:], t2[:], mybir.AluOpType.subtract)

            # ---------------- phase 2 prep: fold GN affine into weights
            # q/k/v = w @ (A*x + B) + b = (w.A) @ x + (w @ B + b); the
            # B-terms are per-output-channel constants computed with tiny
            # N=1 matmuls, then the big matmuls read x_bf directly.
            Bc_bf = stat.tile([P, NCO], BF16)
            nc.vector.tensor_copy(Bc_bf[:], Bc[:])
            kbias = stat.tile([P, NCO], F32)
            qbias = stat.tile([P, NCO], F32)
            for w_sb, b_sb, bias_col in (
                (wkt_sb, bk_sb, kbias),
                (wqt_sb, bq_sb, qbias),
            ):
                for o in range(NCO):
                    tps = psA.tile([P, IB], F32, tag="ps", name=f"tps_{o}")
                    for ci in range(NCO):
                        nc.tensor.matmul(
                            tps[:, 0:1],
                            w_sb[:, ci, o * P : (o + 1) * P],
                            Bc_bf[:, ci : ci + 1],
                            start=(ci == 0), stop=(ci == NCO - 1),
                        )
                    nc.vector.tensor_add(
                        bias_col[:, o : o + 1], tps[:, 0:1], b_sb[:, o : o + 1]
                    )
            # r[c] = B @ wvT, broadcast over partitions, + bv broadcast
            rps = psA.tile([P, IB], F32, tag="ps")
            for ci in range(NCO):
                nc.tensor.matmul(
                    rps[:1, :],
                    Bc_bf[:, ci : ci + 1],
                    wvt_sb[:, ci, :],
                    start=(ci == 0), stop=(ci == NCO - 1),
                )
            r_bf = stat.tile([1, C], BF16)
            nc.vector.tensor_copy(r_bf[:], rps[:1, :])
            vbps = psA.tile([P, IB], F32, tag="ps")
            nc.tensor.matmul(
                vbps[:, :], ones_bf[0:1, :], r_bf[:], start=True, stop=True
            )
            vbias = stat.tile([P, C], F32)
            nc.vector.tensor_add(vbias[:], vbps[:], bvb_sb[:])
            def scale_w(w_sb, name):
                # w' = w * A (per input channel = per partition), new tile so
                # the unscaled-weight bias matmuls don't serialize against it
                w_s = kqv.tile([P, NCO, C], BF16, name=name)
                for ci in range(NCO):
                    nc.vector.tensor_scalar_mul(
                        w_s[:, ci, :], w_sb[:, ci, :], A[:, ci : ci + 1]
                    )
                return w_s

            # ---------------- phase 2: k, vT, q straight from x_bf
            k_sb = kqv.tile([P, NCO, HW], BF16)
            q_sb = kqv.tile([P, NCO, IHALF], BF16)
            vT_sb = kqv.tile([P, NJC, C], BF16)
            wkt_s = scale_w(wkt_sb, "wkt_s")
            for jb in range(NJB):
                js, je = jb * JBLK, (jb + 1) * JBLK
                for o in range(NCO):
                    kps = psA.tile([P, IB], F32, tag="ps")
                    for ci in range(NCO):
                        nc.tensor.matmul(
                            kps[:],
                            wkt_s[:, ci, o * P : (o + 1) * P],
                            x_bf[:, ci, js:je],
                            start=(ci == 0), stop=(ci == NCO - 1),
                        )
                    nc.scalar.add(k_sb[:, o, js:je], kps[:], kbias[:, o : o + 1])
            wvt_s = scale_w(wvt_sb, "wvt_s")
            for jb in range(NJB):
                js, je = jb * JBLK, (jb + 1) * JBLK
                for jc in range(JBLK // P):
                    vps = psA.tile([P, IB], F32, tag="ps")
                    for ci in range(NCO):
                        nc.tensor.matmul(
                            vps[:],
                            x_bf[:, ci, js + jc * P : js + (jc + 1) * P],
                            wvt_s[:, ci, :],
                            start=(ci == 0), stop=(ci == NCO - 1),
                        )
                    jg = jb * (JBLK // P) + jc
                    nc.vector.tensor_add(vT_sb[:, jg, :], vps[:], vbias[:])
            wqt_s = scale_w(wqt_sb, "wqt_s")
            for jb in range(NJB // 2):
                js, je = jb * JBLK, (jb + 1) * JBLK
                for o in range(NCO):
                    qps = psA.tile([P, IB], F32, tag="ps")
                    for ci in range(NCO):
                        nc.tensor.matmul(
                            qps[:],
                            wqt_s[:, ci, o * P : (o + 1) * P],
                            x_bf[:, ci, js:je],
                            start=(ci == 0), stop=(ci == NCO - 1),
                        )
                    nc.scalar.add(q_sb[:, o, js:je], qps[:], qbias[:, o : o + 1])

            # ---------------- phase 4: attention + proj + residual per i-block
            pending = []
            for ib in range(NIB):
                ibs, ibe = ib * IB, (ib + 1) * IB
                u_ps = [
                    psU.tile([P, IB], F32, tag="u", name=f"u_{ib}_{co}")
                    for co in range(NCO)
                ]
                d_ps = psD.tile([P, IB], F32, tag="d")

                def attnv(jg, ex):
                    for co in range(NCO):
                        nc.tensor.matmul(
                            u_ps[co],
                            vT_sb[:, jg, co * P : (co + 1) * P],
                            ex[:],
                            start=(jg == 0), stop=(jg == NJC - 1),
                        )
                    nc.tensor.matmul(
                        d_ps[:], ones_bf[:], ex[:],
                        start=(jg == 0), stop=(jg == NJC - 1),
                    )

                prev = None
                for jg in range(NJC):
                    sps = psA.tile([P, IB], F32, tag="ps")
                    for ci in range(NCO):
                        nc.tensor.matmul(
                            sps[:],
                            k_sb[:, ci, jg * P : (jg + 1) * P],
                            q_sb[:, ci, ibs:ibe],
                            start=(ci == 0), stop=(ci == NCO - 1),
                        )
                    ex = expp.tile([P, IB], BF16, tag="ex")
                    nc.scalar.activation(
                        ex[:], sps[:], mybir.ActivationFunctionType.Exp,
                        bias=0.0, scale=SCALE,
                    )
                    if prev is not None:
                        attnv(*prev)
                    prev = (jg, ex)
                attnv(*prev)

                u_sb = usb.tile([P, NCO, IB], BF16, tag="u_sb")
                for co in range(NCO):
                    nc.vector.tensor_copy(u_sb[:, co, :], u_ps[co])
                drec = drp.tile([P, IB], F32, tag="dr")
                nc.vector.reciprocal(drec[:], d_ps[:])
                x_blk = blk.tile([P, NCO, JBLK], F32, tag="xblk")
                nc.sync.dma_start(x_blk[:], x_r[:, :, ibs:ibe])
                for co in range(NCO):
                    nc.scalar.add(x_blk[:, co, :], x_blk[:, co, :], bp_sb[:, co : co + 1])

                def proj_epilogue(ibs=ibs, ibe=ibe, u_sb=u_sb, drec=drec, x_blk=x_blk):
                    out_sb = osb.tile([P, NCO, IB], F32, tag="out_sb")
                    for o in range(NCO):
                        pps = psA.tile([P, IB], F32, tag="ps", name=f"pps_{ibs}_{o}")
                        for ci in range(NCO):
                            nc.tensor.matmul(
                                pps[:],
                                wpt_sb[:, ci, o * P : (o + 1) * P],
                                u_sb[:, ci, :],
                                start=(ci == 0), stop=(ci == NCO - 1),
                            )
                        nc.vector.tensor_mul(out_sb[:, o, :], pps[:], drec[:])
                        nc.vector.tensor_add(
                            out_sb[:, o, :], out_sb[:, o, :], x_blk[:, o, :]
                        )
                        nc.sync.dma_start(out_r[:, o, ibs:ibe], out_sb[:, o, :])

                # defer this block's proj+epilogue until the next block's
                # attention loop is emitted so PE has ready work at the seam
                pending.append(proj_epilogue)
                if len(pending) > 1:
                    pending.pop(0)()
            for fn in pending:
                fn()

    _split_multi_waits(nc)
    return nc


_NC_CACHE = []


def _get_nc():
    if not _NC_CACHE:
        _NC_CACHE.append(build_bass())
    return _NC_CACHE[0]


def _chunk_pc(v):
    """[512] per-channel vector -> [128, 4] (partition, chunk) layout."""
    return np.ascontiguousarray(v.reshape(NCO, P).T.astype(np.float32))


def kernel(x, gn_scale, gn_bias, wq, bq, wk, bk, wv, bv, wproj, bproj):
    x = np.asarray(x, dtype=np.float32)
    nc = _get_nc()

    aggm = np.zeros((P, 8), np.float32)
    for gg in range(8):
        aggm[gg * 16 : (gg + 1) * 16, gg] = 1.0 / 16.0
    bcm = np.zeros((8, P), np.float32)
    for gg in range(8):
        bcm[gg, gg * 16 : (gg + 1) * 16] = 1.0
    common = {
        "wqt": np.ascontiguousarray(np.asarray(wq, np.float32).T).astype(ml_dtypes.bfloat16),
        "wkt": np.ascontiguousarray(np.asarray(wk, np.float32).T).astype(ml_dtypes.bfloat16),
        "wvt": np.ascontiguousarray(np.asarray(wv, np.float32).T).astype(ml_dtypes.bfloat16),
        "wpt": np.ascontiguousarray(np.asarray(wproj, np.float32).T).astype(ml_dtypes.bfloat16),
        "bq": _chunk_pc(np.asarray(bq)),
        "bk": _chunk_pc(np.asarray(bk)),
        "bp": _chunk_pc(np.asarray(bproj)),
        "bvb": np.ascontiguousarray(np.tile(np.asarray(bv, np.float32)[None, :], (P, 1))),
        "gns": _chunk_pc(np.asarray(gn_scale)),
        "gnb": _chunk_pc(np.asarray(gn_bias)),
        "aggm": aggm,
        "bcm": bcm,
    }
    in_maps = []
    for r in range(8):
        s, h = r // 2, r % 2
        xs = x[s].reshape(C, HW)
        x_rot = np.ascontiguousarray(np.roll(xs, -h * IHALF, axis=1))
        in_maps.append({
            "x": x_rot,
            "xh": x_rot.astype(ml_dtypes.bfloat16),
            **common,
        })

    res = run_bass_kernel_spmd(nc, in_maps, core_ids=list(range(8)))

    out = np.empty((B, C, HW), np.float32)
    for r in range(8):
        s, h = r // 2, r % 2
        out[s][:, h * IHALF : (h + 1) * IHALF] = res.results[r]["out"]
    return out.reshape(B, C, H, W)



# revision 28
# speedup vs baseline: 1.8994x; 1.7850x over previous
"""AttnBlock (GroupNorm + single-head spatial attention + proj + residual)
on 8 Trainium2 NeuronCores via Bass/Tile.

Sharding: batch b=4 -> 4 samples x 2 cores each. Each core receives its
sample's x with its query-half columns rotated to the front (attention is
permutation-invariant over key positions), computes GroupNorm + k + v for
the full sample (redundant with its pair core) and q/attention/proj for its
2048 query positions. No cross-core communication.
"""

import numpy as np
import ml_dtypes

import concourse.bass as bass
import concourse.tile as tile
import concourse.mybir as mybir
from concourse.bass_utils import run_bass_kernel_spmd
from concourse.vector_clock import ScopedClock, VectorClock
from concourse.tile_scheduler import N_PROCS

# ---------------------------------------------------------------- constants
B, C, H, W = 4, 512, 64, 64
HW = H * W            # 4096
P = 128
NCO = C // P          # 4 channel chunks of 128
G = 32                # groups
IHALF = HW // 2       # 2048 query columns per core
IB = 512              # i-block width
NIB = IHALF // IB     # 4
JBLK = 512            # column block for GN/qkv phases
NJB = HW // JBLK      # 8
NJC = HW // P         # 32 j-chunks of 128
EPS = 1e-6
SCALE = float(1.0 / np.sqrt(C))
F32 = mybir.dt.float32
BF16 = mybir.dt.bfloat16
FP8 = mybir.dt.float8e4


# ------------------------------------------------- walrus single-wait fixes
class _TileContextFix(tile.TileContext):
    """TileContext whose tail drain splits sem waits across NOPs.

    The walrus build here rejects instructions carrying more than one sync
    wait ("Too many sync wait commands"), so the stock tail drain (one wait
    per outstanding proc) cannot codegen. Emit one single-wait NOP per proc
    before a wait-free drain.
    """

    def _drain_and_barrier(self, tick_clock, wait_clock):
        gc = tick_clock.global_clock
        for p in range(N_PROCS):
            if gc[p] == 0:
                continue
            partial = VectorClock([gc[q] if q == p else 0 for q in range(N_PROCS)])
            nop_inst = self.nc.sync.nop(nofuse=True, hint=f"tail_wait_{p}")
            wait_clock.add_sem_waits(nop_inst.ins, ScopedClock({None: partial}))
        self.nc.sync.drain()
        self.nc.all_engine_barrier()
        assert self.sems is not None
        popped = self.nc._tile_sem_poison_stack.pop()
        assert popped is self._sem_poison
        self.nc.clear_and_free_semaphores(list(self.sems.allocated().values()))


def _split_multi_waits(nc):
    """Split any instruction with N>1 sync waits into N-1 single-wait NOPs
    prepended on the same engine (same stream -> same ordering; sems are
    monotonic so waiting earlier is safe)."""
    fn = nc.m.functions[0]
    n_split = 0
    for bb in fn.blocks:
        insts = list(bb.instructions)
        out = []
        for inst in insts:
            si = inst.sync_info
            if si is not None and si.on_wait and len(si.on_wait) > 1:
                waits = list(si.on_wait)
                for w in waits[:-1]:
                    nop = mybir.InstNoOp(
                        name=nc.get_next_instruction_name(),
                        engine=inst.engine,
                        sync_info=mybir.SyncInfo(on_wait=[w], on_update=[]),
                        bass_nofuse=True,
                        ins=[],
                        outs=[],
                    )
                    out.append(nop)
                    n_split += 1
                inst.sync_info = mybir.SyncInfo(
                    on_wait=[waits[-1]], on_update=list(si.on_update or [])
                )
            out.append(inst)
        if len(out) != len(insts):
            bb.instructions[:] = out
    return n_split


# ------------------------------------------------------------- the kernel
def build_bass():
    nc = bass.Bass("TRN2", target_bir_lowering=False, debug=False, num_devices=8)

    x_d = nc.dram_tensor("x", [C, HW], F32, kind="ExternalInput")
    xh_d = nc.dram_tensor("xh", [C, HW], BF16, kind="ExternalInput")
    wqt_d = nc.dram_tensor("wqt", [C, C], BF16, kind="ExternalInput")
    wkt_d = nc.dram_tensor("wkt", [C, C], BF16, kind="ExternalInput")
    wvt_d = nc.dram_tensor("wvt", [C, C], BF16, kind="ExternalInput")
    wpt_d = nc.dram_tensor("wpt", [C, C], BF16, kind="ExternalInput")
    bq_d = nc.dram_tensor("bq", [P, NCO], F32, kind="ExternalInput")
    bk_d = nc.dram_tensor("bk", [P, NCO], F32, kind="ExternalInput")
    bp_d = nc.dram_tensor("bp", [P, NCO], F32, kind="ExternalInput")
    bvb_d = nc.dram_tensor("bvb", [P, C], F32, kind="ExternalInput")
    gns_d = nc.dram_tensor("gns", [P, NCO], F32, kind="ExternalInput")
    gnb_d = nc.dram_tensor("gnb", [P, NCO], F32, kind="ExternalInput")
    aggm_d = nc.dram_tensor("aggm", [P, 8], F32, kind="ExternalInput")
    bcm_d = nc.dram_tensor("bcm", [8, P], F32, kind="ExternalInput")
    out_d = nc.dram_tensor("out", [C, IHALF], F32, kind="ExternalOutput")

    x_r = x_d.ap().rearrange("(co p) j -> p co j", p=P)        # [128,4,4096]
    xh_r = xh_d.ap().rearrange("(co p) j -> p co j", p=P)
    out_r = out_d.ap().rearrange("(co p) i -> p co i", p=P)    # [128,4,2048]

    with _TileContextFix(nc) as tc:
        with (
            tc.tile_pool(name="consts", bufs=1) as consts,
            tc.tile_pool(name="xbf", bufs=1) as xbf,
            tc.tile_pool(name="blk", bufs=3) as blk,
            tc.tile_pool(name="kqv", bufs=1) as kqv,
            tc.tile_pool(name="stat", bufs=1) as stat,
            tc.tile_pool(name="expp", bufs=6) as expp,
            tc.tile_pool(name="usb", bufs=2) as usb,
            tc.tile_pool(name="drp", bufs=2) as drp,
            tc.tile_pool(name="osb", bufs=2) as osb,
            tc.tile_pool(name="psA", bufs=3, space="PSUM") as psA,
            tc.tile_pool(name="psU", bufs=4, space="PSUM") as psU,
            tc.tile_pool(name="psD", bufs=1, space="PSUM") as psD,
        ):
            # ---------------- phase 1 loads first (off the weight queues)
            x_bf = xbf.tile([P, NCO, HW], BF16)
            for jb in (6, 7, 0, 1, 2, 3, 4, 5):
                js, je = jb * JBLK, (jb + 1) * JBLK
                eng = nc.gpsimd if jb >= 6 else nc.sync
                eng.dma_start(x_bf[:, :, js:je], xh_r[:, :, js:je])

            # ---------------- constants
            bq_sb = consts.tile([P, NCO], F32)
            nc.sync.dma_start(bq_sb[:], bq_d.ap())
            bk_sb = consts.tile([P, NCO], F32)
            nc.sync.dma_start(bk_sb[:], bk_d.ap())
            bp_sb = consts.tile([P, NCO], F32)
            nc.sync.dma_start(bp_sb[:], bp_d.ap())
            bvb_sb = consts.tile([P, C], F32)
            nc.sync.dma_start(bvb_sb[:], bvb_d.ap())
            gns_sb = consts.tile([P, NCO], F32)
            nc.sync.dma_start(gns_sb[:], gns_d.ap())
            gnb_sb = consts.tile([P, NCO], F32)
            nc.sync.dma_start(gnb_sb[:], gnb_d.ap())
            aggm_sb = consts.tile([P, 8], F32)
            nc.sync.dma_start(aggm_sb[:], aggm_d.ap())
            bcm_sb = consts.tile([8, P], F32)
            nc.sync.dma_start(bcm_sb[:], bcm_d.ap())
            wqt_sb = consts.tile([P, NCO, C], BF16)
            nc.sync.dma_start(wqt_sb[:], wqt_d.ap().rearrange("(ci p) o -> p ci o", p=P))
            wkt_sb = consts.tile([P, NCO, C], BF16)
            nc.sync.dma_start(wkt_sb[:], wkt_d.ap().rearrange("(ci p) o -> p ci o", p=P))
            wvt_sb = consts.tile([P, NCO, C], BF16)
            nc.sync.dma_start(wvt_sb[:], wvt_d.ap().rearrange("(ci p) o -> p ci o", p=P))
            wpt_sb = consts.tile([P, NCO, C], BF16)
            nc.sync.dma_start(wpt_sb[:], wpt_d.ap().rearrange("(ci p) o -> p ci o", p=P))
            ones_bf = consts.tile([P, P], BF16)
            nc.vector.memset(ones_bf[:], 1.0)
            ones8 = consts.tile([P, 2, P], FP8)
            nc.vector.memset(ones8[:], 1.0)
            eps_sb = consts.tile([8, 1], F32)
            nc.vector.memset(eps_sb[:], EPS)

            DVE_BLKS = [0, 1, 2, 3, 4, 5]
            ACT_BLKS = [6, 7]
            stats = stat.tile([P, NCO, len(DVE_BLKS), 6], F32)
            asum = stat.tile([P, NCO, 2, 2], F32)
            mv = stat.tile([P, NCO, 2], F32)

            # ---------------- phase 1: per-channel stats (DVE + ACT split)
            for bi, jb in enumerate(DVE_BLKS):
                js, je = jb * JBLK, (jb + 1) * JBLK
                for co in range(NCO):
                    nc.vector.bn_stats(stats[:, co, bi, :], x_bf[:, co, js:je])
            scr = stat.tile([P, JBLK], BF16)
            for bi, jb in enumerate(ACT_BLKS):
                js, je = jb * JBLK, (jb + 1) * JBLK
                for co in range(NCO):
                    nc.scalar.activation(
                        scr[:], x_bf[:, co, js:je],
                        mybir.ActivationFunctionType.Identity,
                        accum_out=asum[:, co, bi, 0:1],
                    )
                    nc.scalar.activation(
                        scr[:], x_bf[:, co, js:je],
                        mybir.ActivationFunctionType.Square,
                        accum_out=asum[:, co, bi, 1:2],
                    )

            # ---------------- phase 3: group stats -> per-channel affine A, B
            for co in range(NCO):
                nc.vector.bn_aggr(mv[:, co, :], stats[:, co, :, :])
            m2 = stat.tile([P, NCO], F32)
            nc.vector.tensor_mul(m2[:], mv[:, :, 0], mv[:, :, 0])
            nc.vector.tensor_add(mv[:, :, 1], mv[:, :, 1], m2[:])  # E[x^2] (DVE blocks)
            # merge ACT-block sums: stat = (stat6 * 3072 + act_sum) / 4096
            n_dve = float(len(DVE_BLKS) * JBLK)
            sum_t = stat.tile([P, NCO], F32)
            nc.vector.tensor_add(sum_t[:], asum[:, :, 0, 0], asum[:, :, 1, 0])
            ssq_t = stat.tile([P, NCO], F32)
            nc.vector.tensor_add(ssq_t[:], asum[:, :, 0, 1], asum[:, :, 1, 1])
            nc.vector.tensor_scalar(
                mv[:, :, 0], mv[:, :, 0], n_dve, None, op0=mybir.AluOpType.mult
            )
            nc.vector.tensor_add(mv[:, :, 0], mv[:, :, 0], sum_t[:])
            nc.vector.tensor_scalar(
                mv[:, :, 0], mv[:, :, 0], 1.0 / HW, None, op0=mybir.AluOpType.mult
            )
            nc.vector.tensor_scalar(
                mv[:, :, 1], mv[:, :, 1], n_dve, None, op0=mybir.AluOpType.mult
            )
            nc.vector.tensor_add(mv[:, :, 1], mv[:, :, 1], ssq_t[:])
            nc.vector.tensor_scalar(
                mv[:, :, 1], mv[:, :, 1], 1.0 / HW, None, op0=mybir.AluOpType.mult
            )
            ps_s = psA.tile([P, IB], F32, tag="ps")
            nc.tensor.matmul(
                ps_s[:8, : NCO * 2],
                aggm_sb[:],
                mv[:].rearrange("p co s -> p (co s)"),
                start=True, stop=True,
            )
            grp = stat.tile([8, NCO, 2], F32)
            nc.vector.tensor_copy(grp[:], ps_s[:8, : NCO * 2])
            g2 = stat.tile([8, NCO], F32)
            nc.vector.tensor_mul(g2[:], grp[:, :, 0], grp[:, :, 0])
            nc.vector.tensor_tensor(
                grp[:, :, 1], grp[:, :, 1], g2[:], mybir.AluOpType.subtract
            )  # var_g
            nc.scalar.activation(
                grp[:, :, 1], grp[:, :, 1], mybir.ActivationFunctionType.Sqrt,
                bias=eps_sb[:], scale=1.0,
            )
            nc.vector.reciprocal(grp[:, :, 1], grp[:, :, 1])  # rstd_g
            ps_b = psA.tile([P, IB], F32, tag="ps")
            nc.tensor.matmul(
                ps_b[:, : NCO * 2],
                bcm_sb[:],
                grp[:].rearrange("g co s -> g (co s)"),
                start=True, stop=True,
            )
            mvb = stat.tile([P, NCO, 2], F32)  # per-channel (mean_g, rstd_g)
            nc.vector.tensor_copy(mvb[:], ps_b[:, : NCO * 2])
            A = stat.tile([P, NCO], F32)
            nc.vector.tensor_mul(A[:], mvb[:, :, 1], gns_sb[:])
            t2 = stat.tile([P, NCO], F32)
            nc.vector.tensor_mul(t2[:], mvb[:, :, 0], A[:])
            Bc = stat.tile([P, NCO], F32)
            nc.vector.tensor_tensor(Bc[:], gnb_sb[:], t2[:], mybir.AluOpType.subtract)

            # ---------------- phase 2 prep: fold GN affine into weights
            # q/k/v = w @ (A*x + B) + b = (w.A) @ x + (w @ B + b); the
            # B-terms are per-output-channel constants computed with tiny
            # N=1 matmuls, then the big matmuls read x_bf directly.
            Bc_bf = stat.tile([P, NCO], BF16)
            nc.vector.tensor_copy(Bc_bf[:], Bc[:])
            kbias = stat.tile([P, NCO], F32)
            qbias = stat.tile([P, NCO], F32)
            for w_sb, b_sb, bias_col in (
                (wkt_sb, bk_sb, kbias),
                (wqt_sb, bq_sb, qbias),
            ):
                for o in range(NCO):
                    tps = psA.tile([P, IB], F32, tag="ps", name=f"tps_{o}")
                    for ci in range(NCO):
                        nc.tensor.matmul(
                            tps[:, 0:1],
                            w_sb[:, ci, o * P : (o + 1) * P],
                            Bc_bf[:, ci : ci + 1],
                            start=(ci == 0), stop=(ci == NCO - 1),
                        )
                    nc.vector.tensor_add(
                        bias_col[:, o : o + 1], tps[:, 0:1], b_sb[:, o : o + 1]
                    )
            # r[c] = B @ wvT, broadcast over partitions, + bv broadcast
            rps = psA.tile([P, IB], F32, tag="ps")
            for ci in range(NCO):
                nc.tensor.matmul(
                    rps[:1, :],
                    Bc_bf[:, ci : ci + 1],
                    wvt_sb[:, ci, :],
                    start=(ci == 0), stop=(ci == NCO - 1),
                )
            r_bf = stat.tile([1, C], BF16)
            nc.vector.tensor_copy(r_bf[:], rps[:1, :])
            vbps = psA.tile([P, IB], F32, tag="ps")
            nc.tensor.matmul(
                vbps[:, :], ones_bf[0:1, :], r_bf[:], start=True, stop=True
            )
            vbias = stat.tile([P, C], F32)
            nc.vector.tensor_add(vbias[:], vbps[:], bvb_sb[:])
            def scale_w(w_sb, name):
                # w' = w * A (per input channel = per partition), new tile so
                # the unscaled-weight bias matmuls don't serialize against it
                w_s = kqv.tile([P, NCO, C], BF16, name=name)
                for ci in range(NCO):
                    nc.vector.tensor_scalar_mul(
                        w_s[:, ci, :], w_sb[:, ci, :], A[:, ci : ci + 1]
                    )
                return w_s

            # ---------------- phase 2: k, vT, q straight from x_bf
            k_sb = kqv.tile([P, NCO, HW], FP8)
            q_sb = kqv.tile([P, NCO, IHALF], FP8)
            vT_sb = kqv.tile([P, NJC, C], FP8)
            wkt_s = scale_w(wkt_sb, "wkt_s")
            for jb in range(NJB):
                js, je = jb * JBLK, (jb + 1) * JBLK
                for o in range(NCO):
                    kps = psA.tile([P, IB], F32, tag="ps")
                    for ci in range(NCO):
                        nc.tensor.matmul(
                            kps[:],
                            wkt_s[:, ci, o * P : (o + 1) * P],
                            x_bf[:, ci, js:je],
                            start=(ci == 0), stop=(ci == NCO - 1),
                        )
                    nc.scalar.add(k_sb[:, o, js:je], kps[:], kbias[:, o : o + 1])
            wvt_s = scale_w(wvt_sb, "wvt_s")
            for jb in range(NJB):
                js, je = jb * JBLK, (jb + 1) * JBLK
                for jc in range(JBLK // P):
                    vps = psA.tile([P, IB], F32, tag="ps")
                    for ci in range(NCO):
                        nc.tensor.matmul(
                            vps[:],
                            x_bf[:, ci, js + jc * P : js + (jc + 1) * P],
                            wvt_s[:, ci, :],
                            start=(ci == 0), stop=(ci == NCO - 1),
                        )
                    jg = jb * (JBLK // P) + jc
                    nc.vector.tensor_add(vT_sb[:, jg, :], vps[:], vbias[:])
            wqt_s = scale_w(wqt_sb, "wqt_s")
            for jb in range(NJB // 2):
                js, je = jb * JBLK, (jb + 1) * JBLK
                for o in range(NCO):
                    qps = psA.tile([P, IB], F32, tag="ps")
                    for ci in range(NCO):
                        nc.tensor.matmul(
                            qps[:],
                            wqt_s[:, ci, o * P : (o + 1) * P],
                            x_bf[:, ci, js:je],
                            start=(ci == 0), stop=(ci == NCO - 1),
                        )
                    nc.scalar.add(q_sb[:, o, js:je], qps[:], qbias[:, o : o + 1])

            # ---------------- phase 4: attention + proj + residual per i-block
            pending = []
            for ib in range(NIB):
                ibs, ibe = ib * IB, (ib + 1) * IB
                u_ps = [
                    psU.tile([P, IB], F32, tag="u", name=f"u_{ib}_{co}")
                    for co in range(NCO)
                ]
                d_ps = psD.tile([P, IB], F32, tag="d")

                NP2 = NJC // 2  # j-chunk pairs for fp8 DoubleRow

                def attnv(t, ex2):
                    # fp8 DoubleRow: one matmul contracts 256 j positions
                    for co in range(NCO):
                        nc.tensor.matmul(
                            u_ps[co],
                            vT_sb[:, 2 * t : 2 * t + 2, co * P : (co + 1) * P],
                            ex2[:],
                            start=(t == 0), stop=(t == NP2 - 1),
                            perf_mode=mybir.MatmulPerfMode.DoubleRow,
                        )
                    nc.tensor.matmul(
                        d_ps[:], ones8[:], ex2[:],
                        start=(t == 0), stop=(t == NP2 - 1),
                        perf_mode=mybir.MatmulPerfMode.DoubleRow,
                    )

                prev = None
                for t in range(NP2):
                    ex2 = expp.tile([P, 2, IB], FP8, tag="ex")
                    for r in range(2):
                        jg = 2 * t + r
                        sps = psA.tile([P, IB], F32, tag="ps")
                        for cu in range(NCO // 2):
                            nc.tensor.matmul(
                                sps[:],
                                k_sb[:, 2 * cu : 2 * cu + 2, jg * P : (jg + 1) * P],
                                q_sb[:, 2 * cu : 2 * cu + 2, ibs:ibe],
                                start=(cu == 0), stop=(cu == NCO // 2 - 1),
                                perf_mode=mybir.MatmulPerfMode.DoubleRow,
                            )
                        nc.scalar.activation(
                            ex2[:, r, :], sps[:], mybir.ActivationFunctionType.Exp,
                            bias=0.0, scale=SCALE,
                        )
                        if r == 0 and prev is not None:
                            attnv(*prev)
                            prev = None
                    prev = (t, ex2)
                attnv(*prev)

                u_sb = usb.tile([P, NCO, IB], BF16, tag="u_sb")
                for co in range(NCO):
                    nc.vector.tensor_copy(u_sb[:, co, :], u_ps[co])
                drec = drp.tile([P, IB], F32, tag="dr")
                nc.vector.reciprocal(drec[:], d_ps[:])
                x_blk = blk.tile([P, NCO, JBLK], F32, tag="xblk")
                nc.sync.dma_start(x_blk[:], x_r[:, :, ibs:ibe])
                for co in range(NCO):
                    nc.scalar.add(x_blk[:, co, :], x_blk[:, co, :], bp_sb[:, co : co + 1])

                def proj_epilogue(ibs=ibs, ibe=ibe, u_sb=u_sb, drec=drec, x_blk=x_blk):
                    out_sb = osb.tile([P, NCO, IB], F32, tag="out_sb")
                    for o in range(NCO):
                        pps = psA.tile([P, IB], F32, tag="ps", name=f"pps_{ibs}_{o}")
                        for ci in range(NCO):
                            nc.tensor.matmul(
                                pps[:],
                                wpt_sb[:, ci, o * P : (o + 1) * P],
                                u_sb[:, ci, :],
                                start=(ci == 0), stop=(ci == NCO - 1),
                            )
                        nc.vector.tensor_mul(out_sb[:, o, :], pps[:], drec[:])
                        nc.vector.tensor_add(
                            out_sb[:, o, :], out_sb[:, o, :], x_blk[:, o, :]
                        )
                        nc.sync.dma_start(out_r[:, o, ibs:ibe], out_sb[:, o, :])

                # defer this block's proj+epilogue until the next block's
                # attention loop is emitted so PE has ready work at the seam
                pending.append(proj_epilogue)
                if len(pending) > 1:
                    pending.pop(0)()
            for fn in pending:
                fn()

    _split_multi_waits(nc)
    return nc


_NC_CACHE = []


def _get_nc():
    if not _NC_CACHE:
        _NC_CACHE.append(build_bass())
    return _NC_CACHE[0]


def _chunk_pc(v):
    """[512] per-channel vector -> [128, 4] (partition, chunk) layout."""
    return np.ascontiguousarray(v.reshape(NCO, P).T.astype(np.float32))


def kernel(x, gn_scale, gn_bias, wq, bq, wk, bk, wv, bv, wproj, bproj):
    x = np.asarray(x, dtype=np.float32)
    nc = _get_nc()

    aggm = np.zeros((P, 8), np.float32)
    for gg in range(8):
        aggm[gg * 16 : (gg + 1) * 16, gg] = 1.0 / 16.0
    bcm = np.zeros((8, P), np.float32)
    for gg in range(8):
        bcm[gg, gg * 16 : (gg + 1) * 16] = 1.0
    common = {
        "wqt": np.ascontiguousarray(np.asarray(wq, np.float32).T).astype(ml_dtypes.bfloat16),
        "wkt": np.ascontiguousarray(np.asarray(wk, np.float32).T).astype(ml_dtypes.bfloat16),
        "wvt": np.ascontiguousarray(np.asarray(wv, np.float32).T).astype(ml_dtypes.bfloat16),
        "wpt": np.ascontiguousarray(np.asarray(wproj, np.float32).T).astype(ml_dtypes.bfloat16),
        "bq": _chunk_pc(np.asarray(bq)),
        "bk": _chunk_pc(np.asarray(bk)),
        "bp": _chunk_pc(np.asarray(bproj)),
        "bvb": np.ascontiguousarray(np.tile(np.asarray(bv, np.float32)[None, :], (P, 1))),
        "gns": _chunk_pc(np.asarray(gn_scale)),
        "gnb": _chunk_pc(np.asarray(gn_bias)),
        "aggm": aggm,
        "bcm": bcm,
    }
    in_maps = []
    for r in range(8):
        s, h = r // 2, r % 2
        xs = x[s].reshape(C, HW)
        x_rot = np.ascontiguousarray(np.roll(xs, -h * IHALF, axis=1))
        in_maps.append({
            "x": x_rot,
            "xh": x_rot.astype(ml_dtypes.bfloat16),
            **common,
        })

    res = run_bass_kernel_spmd(nc, in_maps, core_ids=list(range(8)))

    out = np.empty((B, C, HW), np.float32)
    for r in range(8):
        s, h = r // 2, r % 2
        out[s][:, h * IHALF : (h + 1) * IHALF] = res.results[r]["out"]
    return out.reshape(B, C, H, W)



# revision 33
# speedup vs baseline: 2.1238x; 1.1181x over previous
"""AttnBlock (GroupNorm + single-head spatial attention + proj + residual)
on 8 Trainium2 NeuronCores via Bass/Tile.

Sharding: batch b=4 -> 4 samples x 2 cores each. Each core receives its
sample's x with its query-half columns rotated to the front (attention is
permutation-invariant over key positions), computes GroupNorm + k + v for
the full sample (redundant with its pair core) and q/attention/proj for its
2048 query positions. No cross-core communication.
"""

import numpy as np
import ml_dtypes

import concourse.bass as bass
import concourse.tile as tile
import concourse.mybir as mybir
from concourse.bass_utils import run_bass_kernel_spmd
from concourse.vector_clock import ScopedClock, VectorClock
from concourse.tile_scheduler import N_PROCS

# ---------------------------------------------------------------- constants
B, C, H, W = 4, 512, 64, 64
HW = H * W            # 4096
P = 128
NCO = C // P          # 4 channel chunks of 128
G = 32                # groups
IHALF = HW // 2       # 2048 query columns per core
IB = 512              # i-block width
NIB = IHALF // IB     # 4
JBLK = 512            # column block for GN/qkv phases
NJB = HW // JBLK      # 8
NJC = HW // P         # 32 j-chunks of 128
EPS = 1e-6
SCALE = float(1.0 / np.sqrt(C))
F32 = mybir.dt.float32
BF16 = mybir.dt.bfloat16
FP8 = mybir.dt.float8e4


# ------------------------------------------------- walrus single-wait fixes
class _TileContextFix(tile.TileContext):
    """TileContext whose tail drain splits sem waits across NOPs.

    The walrus build here rejects instructions carrying more than one sync
    wait ("Too many sync wait commands"), so the stock tail drain (one wait
    per outstanding proc) cannot codegen. Emit one single-wait NOP per proc
    before a wait-free drain.
    """

    def _drain_and_barrier(self, tick_clock, wait_clock):
        gc = tick_clock.global_clock
        for p in range(N_PROCS):
            if gc[p] == 0:
                continue
            partial = VectorClock([gc[q] if q == p else 0 for q in range(N_PROCS)])
            nop_inst = self.nc.sync.nop(nofuse=True, hint=f"tail_wait_{p}")
            wait_clock.add_sem_waits(nop_inst.ins, ScopedClock({None: partial}))
        self.nc.sync.drain()
        self.nc.all_engine_barrier()
        assert self.sems is not None
        popped = self.nc._tile_sem_poison_stack.pop()
        assert popped is self._sem_poison
        self.nc.clear_and_free_semaphores(list(self.sems.allocated().values()))


def _split_multi_waits(nc):
    """Split any instruction with N>1 sync waits into N-1 single-wait NOPs
    prepended on the same engine (same stream -> same ordering; sems are
    monotonic so waiting earlier is safe)."""
    fn = nc.m.functions[0]
    n_split = 0
    for bb in fn.blocks:
        insts = list(bb.instructions)
        out = []
        for inst in insts:
            si = inst.sync_info
            if si is not None and si.on_wait and len(si.on_wait) > 1:
                waits = list(si.on_wait)
                for w in waits[:-1]:
                    nop = mybir.InstNoOp(
                        name=nc.get_next_instruction_name(),
                        engine=inst.engine,
                        sync_info=mybir.SyncInfo(on_wait=[w], on_update=[]),
                        bass_nofuse=True,
                        ins=[],
                        outs=[],
                    )
                    out.append(nop)
                    n_split += 1
                inst.sync_info = mybir.SyncInfo(
                    on_wait=[waits[-1]], on_update=list(si.on_update or [])
                )
            out.append(inst)
        if len(out) != len(insts):
            bb.instructions[:] = out
    return n_split


# ------------------------------------------------------------- the kernel
def build_bass():
    nc = bass.Bass("TRN2", target_bir_lowering=False, debug=False, num_devices=8)

    x_d = nc.dram_tensor("x", [C, HW], F32, kind="ExternalInput")
    xh_d = nc.dram_tensor("xh", [C, HW], BF16, kind="ExternalInput")
    x8_d = nc.dram_tensor("x8", [C, HW], FP8, kind="ExternalInput")
    wqt_d = nc.dram_tensor("wqt", [C, C], BF16, kind="ExternalInput")
    wkt_d = nc.dram_tensor("wkt", [C, C], BF16, kind="ExternalInput")
    wvt_d = nc.dram_tensor("wvt", [C, C], BF16, kind="ExternalInput")
    wpt_d = nc.dram_tensor("wpt", [C, C], BF16, kind="ExternalInput")
    bq_d = nc.dram_tensor("bq", [P, NCO], F32, kind="ExternalInput")
    bk_d = nc.dram_tensor("bk", [P, NCO], F32, kind="ExternalInput")
    bp_d = nc.dram_tensor("bp", [P, NCO], F32, kind="ExternalInput")
    bvb_d = nc.dram_tensor("bvb", [P, C], F32, kind="ExternalInput")
    gns_d = nc.dram_tensor("gns", [P, NCO], F32, kind="ExternalInput")
    gnb_d = nc.dram_tensor("gnb", [P, NCO], F32, kind="ExternalInput")
    aggm_d = nc.dram_tensor("aggm", [P, 8], F32, kind="ExternalInput")
    bcm_d = nc.dram_tensor("bcm", [8, P], F32, kind="ExternalInput")
    out_d = nc.dram_tensor("out", [C, IHALF], F32, kind="ExternalOutput")

    x_r = x_d.ap().rearrange("(co p) j -> p co j", p=P)        # [128,4,4096]
    xh_r = xh_d.ap().rearrange("(co p) j -> p co j", p=P)
    x8_r = x8_d.ap().rearrange("(co p) j -> p co j", p=P)
    out_r = out_d.ap().rearrange("(co p) i -> p co i", p=P)    # [128,4,2048]

    with _TileContextFix(nc) as tc:
        with (
            tc.tile_pool(name="consts", bufs=1) as consts,
            tc.tile_pool(name="xbf", bufs=1) as xbf,
            tc.tile_pool(name="blk", bufs=3) as blk,
            tc.tile_pool(name="kqv", bufs=1) as kqv,
            tc.tile_pool(name="stat", bufs=1) as stat,
            tc.tile_pool(name="expp", bufs=6) as expp,
            tc.tile_pool(name="usb", bufs=2) as usb,
            tc.tile_pool(name="drp", bufs=2) as drp,
            tc.tile_pool(name="osb", bufs=2) as osb,
            tc.tile_pool(name="psA", bufs=3, space="PSUM") as psA,
            tc.tile_pool(name="psU", bufs=4, space="PSUM") as psU,
            tc.tile_pool(name="psD", bufs=1, space="PSUM") as psD,
        ):
            # ---------------- phase 1 loads first (off the weight queues)
            x_bf = xbf.tile([P, NCO, HW], BF16)
            for jb in (6, 7, 0, 1, 2, 3, 4, 5):
                js, je = jb * JBLK, (jb + 1) * JBLK
                eng = nc.gpsimd if jb >= 6 else nc.sync
                eng.dma_start(x_bf[:, :, js:je], xh_r[:, :, js:je])
            x8_sb = xbf.tile([P, NCO, HW], FP8)
            nc.gpsimd.dma_start(x8_sb[:], x8_r)

            # ---------------- constants
            bq_sb = consts.tile([P, NCO], F32)
            nc.sync.dma_start(bq_sb[:], bq_d.ap())
            bk_sb = consts.tile([P, NCO], F32)
            nc.sync.dma_start(bk_sb[:], bk_d.ap())
            bp_sb = consts.tile([P, NCO], F32)
            nc.sync.dma_start(bp_sb[:], bp_d.ap())
            bvb_sb = consts.tile([P, C], F32)
            nc.sync.dma_start(bvb_sb[:], bvb_d.ap())
            gns_sb = consts.tile([P, NCO], F32)
            nc.sync.dma_start(gns_sb[:], gns_d.ap())
            gnb_sb = consts.tile([P, NCO], F32)
            nc.sync.dma_start(gnb_sb[:], gnb_d.ap())
            aggm_sb = consts.tile([P, 8], F32)
            nc.sync.dma_start(aggm_sb[:], aggm_d.ap())
            bcm_sb = consts.tile([8, P], F32)
            nc.sync.dma_start(bcm_sb[:], bcm_d.ap())
            wqt_sb = consts.tile([P, NCO, C], BF16)
            nc.sync.dma_start(wqt_sb[:], wqt_d.ap().rearrange("(ci p) o -> p ci o", p=P))
            wkt_sb = consts.tile([P, NCO, C], BF16)
            nc.sync.dma_start(wkt_sb[:], wkt_d.ap().rearrange("(ci p) o -> p ci o", p=P))
            wvt_sb = consts.tile([P, NCO, C], BF16)
            nc.sync.dma_start(wvt_sb[:], wvt_d.ap().rearrange("(ci p) o -> p ci o", p=P))
            wpt_sb = consts.tile([P, NCO, C], BF16)
            nc.sync.dma_start(wpt_sb[:], wpt_d.ap().rearrange("(ci p) o -> p ci o", p=P))
            ones_bf = consts.tile([P, P], BF16)
            nc.vector.memset(ones_bf[:], 1.0)
            ones8 = consts.tile([P, 2, P], FP8)
            nc.vector.memset(ones8[:], 1.0)
            eps_sb = consts.tile([8, 1], F32)
            nc.vector.memset(eps_sb[:], EPS)

            DVE_BLKS = [0, 1, 2, 3, 4, 5]
            ACT_BLKS = [6, 7]
            stats = stat.tile([P, NCO, len(DVE_BLKS), 6], F32)
            asum = stat.tile([P, NCO, 2, 2], F32)
            mv = stat.tile([P, NCO, 2], F32)

            # ---------------- phase 1: per-channel stats (DVE + ACT split)
            for bi, jb in enumerate(DVE_BLKS):
                js, je = jb * JBLK, (jb + 1) * JBLK
                for co in range(NCO):
                    nc.vector.bn_stats(stats[:, co, bi, :], x_bf[:, co, js:je])
            scr = stat.tile([P, JBLK], BF16)
            for bi, jb in enumerate(ACT_BLKS):
                js, je = jb * JBLK, (jb + 1) * JBLK
                for co in range(NCO):
                    nc.scalar.activation(
                        scr[:], x_bf[:, co, js:je],
                        mybir.ActivationFunctionType.Identity,
                        accum_out=asum[:, co, bi, 0:1],
                    )
                    nc.scalar.activation(
                        scr[:], x_bf[:, co, js:je],
                        mybir.ActivationFunctionType.Square,
                        accum_out=asum[:, co, bi, 1:2],
                    )

            # ---------------- phase 3: group stats -> per-channel affine A, B
            for co in range(NCO):
                nc.vector.bn_aggr(mv[:, co, :], stats[:, co, :, :])
            m2 = stat.tile([P, NCO], F32)
            nc.vector.tensor_mul(m2[:], mv[:, :, 0], mv[:, :, 0])
            nc.vector.tensor_add(mv[:, :, 1], mv[:, :, 1], m2[:])  # E[x^2] (DVE blocks)
            # merge ACT-block sums: stat = (stat6 * 3072 + act_sum) / 4096
            n_dve = float(len(DVE_BLKS) * JBLK)
            sum_t = stat.tile([P, NCO], F32)
            nc.vector.tensor_add(sum_t[:], asum[:, :, 0, 0], asum[:, :, 1, 0])
            ssq_t = stat.tile([P, NCO], F32)
            nc.vector.tensor_add(ssq_t[:], asum[:, :, 0, 1], asum[:, :, 1, 1])
            nc.vector.tensor_scalar(
                mv[:, :, 0], mv[:, :, 0], n_dve, None, op0=mybir.AluOpType.mult
            )
            nc.vector.tensor_add(mv[:, :, 0], mv[:, :, 0], sum_t[:])
            nc.vector.tensor_scalar(
                mv[:, :, 0], mv[:, :, 0], 1.0 / HW, None, op0=mybir.AluOpType.mult
            )
            nc.vector.tensor_scalar(
                mv[:, :, 1], mv[:, :, 1], n_dve, None, op0=mybir.AluOpType.mult
            )
            nc.vector.tensor_add(mv[:, :, 1], mv[:, :, 1], ssq_t[:])
            nc.vector.tensor_scalar(
                mv[:, :, 1], mv[:, :, 1], 1.0 / HW, None, op0=mybir.AluOpType.mult
            )
            ps_s = psA.tile([P, IB], F32, tag="ps")
            nc.tensor.matmul(
                ps_s[:8, : NCO * 2],
                aggm_sb[:],
                mv[:].rearrange("p co s -> p (co s)"),
                start=True, stop=True,
            )
            grp = stat.tile([8, NCO, 2], F32)
            nc.vector.tensor_copy(grp[:], ps_s[:8, : NCO * 2])
            g2 = stat.tile([8, NCO], F32)
            nc.vector.tensor_mul(g2[:], grp[:, :, 0], grp[:, :, 0])
            nc.vector.tensor_tensor(
                grp[:, :, 1], grp[:, :, 1], g2[:], mybir.AluOpType.subtract
            )  # var_g
            nc.scalar.activation(
                grp[:, :, 1], grp[:, :, 1], mybir.ActivationFunctionType.Sqrt,
                bias=eps_sb[:], scale=1.0,
            )
            nc.vector.reciprocal(grp[:, :, 1], grp[:, :, 1])  # rstd_g
            ps_b = psA.tile([P, IB], F32, tag="ps")
            nc.tensor.matmul(
                ps_b[:, : NCO * 2],
                bcm_sb[:],
                grp[:].rearrange("g co s -> g (co s)"),
                start=True, stop=True,
            )
            mvb = stat.tile([P, NCO, 2], F32)  # per-channel (mean_g, rstd_g)
            nc.vector.tensor_copy(mvb[:], ps_b[:, : NCO * 2])
            A = stat.tile([P, NCO], F32)
            nc.vector.tensor_mul(A[:], mvb[:, :, 1], gns_sb[:])
            t2 = stat.tile([P, NCO], F32)
            nc.vector.tensor_mul(t2[:], mvb[:, :, 0], A[:])
            Bc = stat.tile([P, NCO], F32)
            nc.vector.tensor_tensor(Bc[:], gnb_sb[:], t2[:], mybir.AluOpType.subtract)

            # ---------------- phase 2 prep: fold GN affine into weights
            # q/k/v = w @ (A*x + B) + b = (w.A) @ x + (w @ B + b); the
            # B-terms are per-output-channel constants computed with tiny
            # N=1 matmuls, then the big matmuls read x_bf directly.
            Bc_bf = stat.tile([P, NCO], BF16)
            nc.vector.tensor_copy(Bc_bf[:], Bc[:])
            kbias = stat.tile([P, NCO], F32)
            qbias = stat.tile([P, NCO], F32)
            for w_sb, b_sb, bias_col in (
                (wkt_sb, bk_sb, kbias),
                (wqt_sb, bq_sb, qbias),
            ):
                for o in range(NCO):
                    tps = psA.tile([P, IB], F32, tag="ps", name=f"tps_{o}")
                    for ci in range(NCO):
                        nc.tensor.matmul(
                            tps[:, 0:1],
                            w_sb[:, ci, o * P : (o + 1) * P],
                            Bc_bf[:, ci : ci + 1],
                            start=(ci == 0), stop=(ci == NCO - 1),
                        )
                    nc.vector.tensor_add(
                        bias_col[:, o : o + 1], tps[:, 0:1], b_sb[:, o : o + 1]
                    )
            # r[c] = B @ wvT, broadcast over partitions, + bv broadcast
            rps = psA.tile([P, IB], F32, tag="ps")
            for ci in range(NCO):
                nc.tensor.matmul(
                    rps[:1, :],
                    Bc_bf[:, ci : ci + 1],
                    wvt_sb[:, ci, :],
                    start=(ci == 0), stop=(ci == NCO - 1),
                )
            r_bf = stat.tile([1, C], BF16)
            nc.vector.tensor_copy(r_bf[:], rps[:1, :])
            vbps = psA.tile([P, IB], F32, tag="ps")
            nc.tensor.matmul(
                vbps[:, :], ones_bf[0:1, :], r_bf[:], start=True, stop=True
            )
            vbias = stat.tile([P, C], F32)
            nc.vector.tensor_add(vbias[:], vbps[:], bvb_sb[:])
            def scale_w(w_sb, name):
                # w' = w * A (per input channel = per partition), new tile so
                # the unscaled-weight bias matmuls don't serialize against it
                w_s = kqv.tile([P, NCO, C], FP8, name=name)
                for ci in range(NCO):
                    nc.vector.tensor_scalar_mul(
                        w_s[:, ci, :], w_sb[:, ci, :], A[:, ci : ci + 1]
                    )
                return w_s

            # ---------------- phase 2: k, vT, q straight from x_bf
            k_sb = kqv.tile([P, NCO, HW], FP8)
            q_sb = kqv.tile([P, NCO, IHALF], FP8)
            vT_sb = kqv.tile([P, NJC, C], FP8)
            wkt_s = scale_w(wkt_sb, "wkt_s")
            for jb in range(NJB):
                js, je = jb * JBLK, (jb + 1) * JBLK
                for o in range(NCO):
                    kps = psA.tile([P, IB], F32, tag="ps")
                    for cu in range(NCO // 2):
                        nc.tensor.matmul(
                            kps[:],
                            wkt_s[:, 2 * cu : 2 * cu + 2, o * P : (o + 1) * P],
                            x8_sb[:, 2 * cu : 2 * cu + 2, js:je],
                            start=(cu == 0), stop=(cu == NCO // 2 - 1),
                            perf_mode=mybir.MatmulPerfMode.DoubleRow,
                        )
                    if (jb + o) % 2 == 0:
                        nc.scalar.add(k_sb[:, o, js:je], kps[:], kbias[:, o : o + 1])
                    else:
                        nc.vector.tensor_scalar(
                            k_sb[:, o, js:je], kps[:], kbias[:, o : o + 1],
                            None, op0=mybir.AluOpType.add,
                        )
            wvt_s = scale_w(wvt_sb, "wvt_s")
            for jb in range(NJB):
                js, je = jb * JBLK, (jb + 1) * JBLK
                for jc in range(JBLK // P):
                    vps = psA.tile([P, IB], F32, tag="ps")
                    for cu in range(NCO // 2):
                        nc.tensor.matmul(
                            vps[:],
                            x8_sb[:, 2 * cu : 2 * cu + 2, js + jc * P : js + (jc + 1) * P],
                            wvt_s[:, 2 * cu : 2 * cu + 2, :],
                            start=(cu == 0), stop=(cu == NCO // 2 - 1),
                            perf_mode=mybir.MatmulPerfMode.DoubleRow,
                        )
                    jg = jb * (JBLK // P) + jc
                    nc.vector.tensor_add(vT_sb[:, jg, :], vps[:], vbias[:])
            wqt_s = scale_w(wqt_sb, "wqt_s")
            for jb in range(NJB // 2):
                js, je = jb * JBLK, (jb + 1) * JBLK
                for o in range(NCO):
                    qps = psA.tile([P, IB], F32, tag="ps")
                    for cu in range(NCO // 2):
                        nc.tensor.matmul(
                            qps[:],
                            wqt_s[:, 2 * cu : 2 * cu + 2, o * P : (o + 1) * P],
                            x8_sb[:, 2 * cu : 2 * cu + 2, js:je],
                            start=(cu == 0), stop=(cu == NCO // 2 - 1),
                            perf_mode=mybir.MatmulPerfMode.DoubleRow,
                        )
                    if (jb + o) % 2 == 0:
                        nc.scalar.add(q_sb[:, o, js:je], qps[:], qbias[:, o : o + 1])
                    else:
                        nc.vector.tensor_scalar(
                            q_sb[:, o, js:je], qps[:], qbias[:, o : o + 1],
                            None, op0=mybir.AluOpType.add,
                        )

            # ---------------- phase 4: attention + proj + residual per i-block
            pending = []
            for ib in range(NIB):
                ibs, ibe = ib * IB, (ib + 1) * IB
                u_ps = [
                    psU.tile([P, IB], F32, tag="u", name=f"u_{ib}_{co}")
                    for co in range(NCO)
                ]
                d_ps = psD.tile([P, IB], F32, tag="d")

                NP2 = NJC // 2  # j-chunk pairs for fp8 DoubleRow

                def attnv(t, ex2):
                    # fp8 DoubleRow: one matmul contracts 256 j positions
                    for co in range(NCO):
                        nc.tensor.matmul(
                            u_ps[co],
                            vT_sb[:, 2 * t : 2 * t + 2, co * P : (co + 1) * P],
                            ex2[:],
                            start=(t == 0), stop=(t == NP2 - 1),
                            perf_mode=mybir.MatmulPerfMode.DoubleRow,
                        )
                    nc.tensor.matmul(
                        d_ps[:], ones8[:], ex2[:],
                        start=(t == 0), stop=(t == NP2 - 1),
                        perf_mode=mybir.MatmulPerfMode.DoubleRow,
                    )

                prev = None
                for t in range(NP2):
                    ex2 = expp.tile([P, 2, IB], FP8, tag="ex")
                    for r in range(2):
                        jg = 2 * t + r
                        sps = psA.tile([P, IB], F32, tag="ps")
                        for cu in range(NCO // 2):
                            nc.tensor.matmul(
                                sps[:],
                                k_sb[:, 2 * cu : 2 * cu + 2, jg * P : (jg + 1) * P],
                                q_sb[:, 2 * cu : 2 * cu + 2, ibs:ibe],
                                start=(cu == 0), stop=(cu == NCO // 2 - 1),
                                perf_mode=mybir.MatmulPerfMode.DoubleRow,
                            )
                        nc.scalar.activation(
                            ex2[:, r, :], sps[:], mybir.ActivationFunctionType.Exp,
                            bias=0.0, scale=SCALE,
                        )
                        if r == 0 and prev is not None:
                            attnv(*prev)
                            prev = None
                    prev = (t, ex2)
                attnv(*prev)

                u_sb = usb.tile([P, NCO, IB], BF16, tag="u_sb")
                for co in range(NCO):
                    nc.vector.tensor_copy(u_sb[:, co, :], u_ps[co])
                drec = drp.tile([P, IB], F32, tag="dr")
                nc.vector.reciprocal(drec[:], d_ps[:])
                x_blk = blk.tile([P, NCO, JBLK], F32, tag="xblk")
                nc.sync.dma_start(x_blk[:], x_r[:, :, ibs:ibe])
                for co in range(NCO):
                    nc.vector.tensor_scalar(
                        x_blk[:, co, :], x_blk[:, co, :], bp_sb[:, co : co + 1],
                        None, op0=mybir.AluOpType.add,
                    )

                def proj_epilogue(ibs=ibs, ibe=ibe, u_sb=u_sb, drec=drec, x_blk=x_blk):
                    out_sb = osb.tile([P, NCO, IB], F32, tag="out_sb")
                    for o in range(NCO):
                        pps = psA.tile([P, IB], F32, tag="ps", name=f"pps_{ibs}_{o}")
                        for ci in range(NCO):
                            nc.tensor.matmul(
                                pps[:],
                                wpt_sb[:, ci, o * P : (o + 1) * P],
                                u_sb[:, ci, :],
                                start=(ci == 0), stop=(ci == NCO - 1),
                            )
                        nc.vector.tensor_mul(out_sb[:, o, :], pps[:], drec[:])
                        nc.vector.tensor_add(
                            out_sb[:, o, :], out_sb[:, o, :], x_blk[:, o, :]
                        )
                        nc.sync.dma_start(out_r[:, o, ibs:ibe], out_sb[:, o, :])

                # defer this block's proj+epilogue until the next block's
                # attention loop is emitted so PE has ready work at the seam
                pending.append(proj_epilogue)
                if len(pending) > 1:
                    pending.pop(0)()
            for fn in pending:
                fn()

    _split_multi_waits(nc)
    return nc


_NC_CACHE = []


def _get_nc():
    if not _NC_CACHE:
        _NC_CACHE.append(build_bass())
    return _NC_CACHE[0]


def _chunk_pc(v):
    """[512] per-channel vector -> [128, 4] (partition, chunk) layout."""
    return np.ascontiguousarray(v.reshape(NCO, P).T.astype(np.float32))


def kernel(x, gn_scale, gn_bias, wq, bq, wk, bk, wv, bv, wproj, bproj):
    x = np.asarray(x, dtype=np.float32)
    nc = _get_nc()

    aggm = np.zeros((P, 8), np.float32)
    for gg in range(8):
        aggm[gg * 16 : (gg + 1) * 16, gg] = 1.0 / 16.0
    bcm = np.zeros((8, P), np.float32)
    for gg in range(8):
        bcm[gg, gg * 16 : (gg + 1) * 16] = 1.0
    common = {
        "wqt": np.ascontiguousarray(np.asarray(wq, np.float32).T).astype(ml_dtypes.bfloat16),
        "wkt": np.ascontiguousarray(np.asarray(wk, np.float32).T).astype(ml_dtypes.bfloat16),
        "wvt": np.ascontiguousarray(np.asarray(wv, np.float32).T).astype(ml_dtypes.bfloat16),
        "wpt": np.ascontiguousarray(np.asarray(wproj, np.float32).T).astype(ml_dtypes.bfloat16),
        "bq": _chunk_pc(np.asarray(bq)),
        "bk": _chunk_pc(np.asarray(bk)),
        "bp": _chunk_pc(np.asarray(bproj)),
        "bvb": np.ascontiguousarray(np.tile(np.asarray(bv, np.float32)[None, :], (P, 1))),
        "gns": _chunk_pc(np.asarray(gn_scale)),
        "gnb": _chunk_pc(np.asarray(gn_bias)),
        "aggm": aggm,
        "bcm": bcm,
    }
    in_maps = []
    for r in range(8):
        s, h = r // 2, r % 2
        xs = x[s].reshape(C, HW)
        x_rot = np.ascontiguousarray(np.roll(xs, -h * IHALF, axis=1))
        in_maps.append({
            "x": x_rot,
            "xh": x_rot.astype(ml_dtypes.bfloat16),
            "x8": x_rot.astype(ml_dtypes.float8_e4m3),
            **common,
        })

    res = run_bass_kernel_spmd(nc, in_maps, core_ids=list(range(8)))

    out = np.empty((B, C, HW), np.float32)
    for r in range(8):
        s, h = r // 2, r % 2
        out[s][:, h * IHALF : (h + 1) * IHALF] = res.results[r]["out"]
    return out.reshape(B, C, H, W)



# revision 36
# speedup vs baseline: 2.1456x; 1.0103x over previous
"""AttnBlock (GroupNorm + single-head spatial attention + proj + residual)
on 8 Trainium2 NeuronCores via Bass/Tile.

Sharding: batch b=4 -> 4 samples x 2 cores each. Each core receives its
sample's x with its query-half columns rotated to the front (attention is
permutation-invariant over key positions), computes GroupNorm + k + v for
the full sample (redundant with its pair core) and q/attention/proj for its
2048 query positions. No cross-core communication.
"""

import numpy as np
import ml_dtypes

import concourse.bass as bass
import concourse.tile as tile
import concourse.mybir as mybir
from concourse.bass_utils import run_bass_kernel_spmd
from concourse.vector_clock import ScopedClock, VectorClock
from concourse.tile_scheduler import N_PROCS

# ---------------------------------------------------------------- constants
B, C, H, W = 4, 512, 64, 64
HW = H * W            # 4096
P = 128
NCO = C // P          # 4 channel chunks of 128
G = 32                # groups
IHALF = HW // 2       # 2048 query columns per core
IB = 512              # i-block width
NIB = IHALF // IB     # 4
JBLK = 512            # column block for GN/qkv phases
NJB = HW // JBLK      # 8
NJC = HW // P         # 32 j-chunks of 128
EPS = 1e-6
SCALE = float(1.0 / np.sqrt(C))
F32 = mybir.dt.float32
BF16 = mybir.dt.bfloat16
FP8 = mybir.dt.float8e4


# ------------------------------------------------- walrus single-wait fixes
class _TileContextFix(tile.TileContext):
    """TileContext whose tail drain splits sem waits across NOPs.

    The walrus build here rejects instructions carrying more than one sync
    wait ("Too many sync wait commands"), so the stock tail drain (one wait
    per outstanding proc) cannot codegen. Emit one single-wait NOP per proc
    before a wait-free drain.
    """

    def _drain_and_barrier(self, tick_clock, wait_clock):
        gc = tick_clock.global_clock
        for p in range(N_PROCS):
            if gc[p] == 0:
                continue
            partial = VectorClock([gc[q] if q == p else 0 for q in range(N_PROCS)])
            nop_inst = self.nc.sync.nop(nofuse=True, hint=f"tail_wait_{p}")
            wait_clock.add_sem_waits(nop_inst.ins, ScopedClock({None: partial}))
        self.nc.sync.drain()
        self.nc.all_engine_barrier()
        assert self.sems is not None
        popped = self.nc._tile_sem_poison_stack.pop()
        assert popped is self._sem_poison
        self.nc.clear_and_free_semaphores(list(self.sems.allocated().values()))


def _split_multi_waits(nc):
    """Split any instruction with N>1 sync waits into N-1 single-wait NOPs
    prepended on the same engine (same stream -> same ordering; sems are
    monotonic so waiting earlier is safe)."""
    fn = nc.m.functions[0]
    n_split = 0
    for bb in fn.blocks:
        insts = list(bb.instructions)
        out = []
        for inst in insts:
            si = inst.sync_info
            if si is not None and si.on_wait and len(si.on_wait) > 1:
                waits = list(si.on_wait)
                for w in waits[:-1]:
                    nop = mybir.InstNoOp(
                        name=nc.get_next_instruction_name(),
                        engine=inst.engine,
                        sync_info=mybir.SyncInfo(on_wait=[w], on_update=[]),
                        bass_nofuse=True,
                        ins=[],
                        outs=[],
                    )
                    out.append(nop)
                    n_split += 1
                inst.sync_info = mybir.SyncInfo(
                    on_wait=[waits[-1]], on_update=list(si.on_update or [])
                )
            out.append(inst)
        if len(out) != len(insts):
            bb.instructions[:] = out
    return n_split


# ------------------------------------------------------------- the kernel
def build_bass():
    nc = bass.Bass("TRN2", target_bir_lowering=False, debug=False, num_devices=8)

    x_d = nc.dram_tensor("x", [C, HW], F32, kind="ExternalInput")
    xh_d = nc.dram_tensor("xh", [C, HW], BF16, kind="ExternalInput")
    x8_d = nc.dram_tensor("x8", [C, HW], FP8, kind="ExternalInput")
    wqt_d = nc.dram_tensor("wqt", [C, C], BF16, kind="ExternalInput")
    wkt_d = nc.dram_tensor("wkt", [C, C], BF16, kind="ExternalInput")
    wvt_d = nc.dram_tensor("wvt", [C, C], BF16, kind="ExternalInput")
    wpt_d = nc.dram_tensor("wpt", [C, C], BF16, kind="ExternalInput")
    bq_d = nc.dram_tensor("bq", [P, NCO], F32, kind="ExternalInput")
    bk_d = nc.dram_tensor("bk", [P, NCO], F32, kind="ExternalInput")
    bp_d = nc.dram_tensor("bp", [P, NCO], F32, kind="ExternalInput")
    bvb_d = nc.dram_tensor("bvb", [P, C], F32, kind="ExternalInput")
    gns_d = nc.dram_tensor("gns", [P, NCO], F32, kind="ExternalInput")
    gnb_d = nc.dram_tensor("gnb", [P, NCO], F32, kind="ExternalInput")
    aggm_d = nc.dram_tensor("aggm", [P, 8], F32, kind="ExternalInput")
    bcm_d = nc.dram_tensor("bcm", [8, P], F32, kind="ExternalInput")
    out_d = nc.dram_tensor("out", [C, IHALF], F32, kind="ExternalOutput")

    x_r = x_d.ap().rearrange("(co p) j -> p co j", p=P)        # [128,4,4096]
    xh_r = xh_d.ap().rearrange("(co p) j -> p co j", p=P)
    x8_r = x8_d.ap().rearrange("(co p) j -> p co j", p=P)
    out_r = out_d.ap().rearrange("(co p) i -> p co i", p=P)    # [128,4,2048]

    with _TileContextFix(nc) as tc:
        with (
            tc.tile_pool(name="consts", bufs=1) as consts,
            tc.tile_pool(name="xbf", bufs=1) as xbf,
            tc.tile_pool(name="blk", bufs=3) as blk,
            tc.tile_pool(name="kqv", bufs=1) as kqv,
            tc.tile_pool(name="stat", bufs=1) as stat,
            tc.tile_pool(name="expp", bufs=6) as expp,
            tc.tile_pool(name="usb", bufs=2) as usb,
            tc.tile_pool(name="drp", bufs=2) as drp,
            tc.tile_pool(name="osb", bufs=2) as osb,
        ):
            psq_ctx = tc.tile_pool(name="psQKV", bufs=6, space="PSUM")
            psA = psq_ctx.__enter__()

            # ---------------- phase 1 loads first (off the weight queues)
            x_bf = xbf.tile([P, NCO, HW], BF16)
            for jb in (6, 7, 0, 1, 2, 3, 4, 5):
                js, je = jb * JBLK, (jb + 1) * JBLK
                eng = nc.gpsimd if jb >= 6 else nc.sync
                eng.dma_start(x_bf[:, :, js:je], xh_r[:, :, js:je])
            x8_sb = xbf.tile([P, NCO, HW], FP8)
            nc.gpsimd.dma_start(x8_sb[:], x8_r)

            # ---------------- constants
            bq_sb = consts.tile([P, NCO], F32)
            nc.sync.dma_start(bq_sb[:], bq_d.ap())
            bk_sb = consts.tile([P, NCO], F32)
            nc.sync.dma_start(bk_sb[:], bk_d.ap())
            bp_sb = consts.tile([P, NCO], F32)
            nc.sync.dma_start(bp_sb[:], bp_d.ap())
            bvb_sb = consts.tile([P, C], F32)
            nc.sync.dma_start(bvb_sb[:], bvb_d.ap())
            gns_sb = consts.tile([P, NCO], F32)
            nc.sync.dma_start(gns_sb[:], gns_d.ap())
            gnb_sb = consts.tile([P, NCO], F32)
            nc.sync.dma_start(gnb_sb[:], gnb_d.ap())
            aggm_sb = consts.tile([P, 8], F32)
            nc.sync.dma_start(aggm_sb[:], aggm_d.ap())
            bcm_sb = consts.tile([8, P], F32)
            nc.sync.dma_start(bcm_sb[:], bcm_d.ap())
            wqt_sb = consts.tile([P, NCO, C], BF16)
            nc.sync.dma_start(wqt_sb[:], wqt_d.ap().rearrange("(ci p) o -> p ci o", p=P))
            wkt_sb = consts.tile([P, NCO, C], BF16)
            nc.sync.dma_start(wkt_sb[:], wkt_d.ap().rearrange("(ci p) o -> p ci o", p=P))
            wvt_sb = consts.tile([P, NCO, C], BF16)
            nc.sync.dma_start(wvt_sb[:], wvt_d.ap().rearrange("(ci p) o -> p ci o", p=P))
            wpt_sb = consts.tile([P, NCO, C], BF16)
            nc.sync.dma_start(wpt_sb[:], wpt_d.ap().rearrange("(ci p) o -> p ci o", p=P))
            ones_bf = consts.tile([P, P], BF16)
            nc.vector.memset(ones_bf[:], 1.0)
            ones8 = consts.tile([P, 2, P], FP8)
            nc.vector.memset(ones8[:], 1.0)
            eps_sb = consts.tile([8, 1], F32)
            nc.vector.memset(eps_sb[:], EPS)

            DVE_BLKS = [0, 1, 2, 3, 4, 5]
            ACT_BLKS = [6, 7]
            stats = stat.tile([P, NCO, len(DVE_BLKS), 6], F32)
            asum = stat.tile([P, NCO, 2, 2], F32)
            mv = stat.tile([P, NCO, 2], F32)

            # ---------------- phase 1: per-channel stats (DVE + ACT split)
            for bi, jb in enumerate(DVE_BLKS):
                js, je = jb * JBLK, (jb + 1) * JBLK
                for co in range(NCO):
                    nc.vector.bn_stats(stats[:, co, bi, :], x_bf[:, co, js:je])
            scr = stat.tile([P, JBLK], BF16)
            for bi, jb in enumerate(ACT_BLKS):
                js, je = jb * JBLK, (jb + 1) * JBLK
                for co in range(NCO):
                    nc.scalar.activation(
                        scr[:], x_bf[:, co, js:je],
                        mybir.ActivationFunctionType.Identity,
                        accum_out=asum[:, co, bi, 0:1],
                    )
                    nc.scalar.activation(
                        scr[:], x_bf[:, co, js:je],
                        mybir.ActivationFunctionType.Square,
                        accum_out=asum[:, co, bi, 1:2],
                    )

            # ---------------- phase 3: group stats -> per-channel affine A, B
            for co in range(NCO):
                nc.vector.bn_aggr(mv[:, co, :], stats[:, co, :, :])
            m2 = stat.tile([P, NCO], F32)
            nc.vector.tensor_mul(m2[:], mv[:, :, 0], mv[:, :, 0])
            nc.vector.tensor_add(mv[:, :, 1], mv[:, :, 1], m2[:])  # E[x^2] (DVE blocks)
            # merge ACT-block sums: stat = (stat6 * 3072 + act_sum) / 4096
            n_dve = float(len(DVE_BLKS) * JBLK)
            sum_t = stat.tile([P, NCO], F32)
            nc.vector.tensor_add(sum_t[:], asum[:, :, 0, 0], asum[:, :, 1, 0])
            ssq_t = stat.tile([P, NCO], F32)
            nc.vector.tensor_add(ssq_t[:], asum[:, :, 0, 1], asum[:, :, 1, 1])
            nc.vector.tensor_scalar(
                mv[:, :, 0], mv[:, :, 0], n_dve, None, op0=mybir.AluOpType.mult
            )
            nc.vector.tensor_add(mv[:, :, 0], mv[:, :, 0], sum_t[:])
            nc.vector.tensor_scalar(
                mv[:, :, 0], mv[:, :, 0], 1.0 / HW, None, op0=mybir.AluOpType.mult
            )
            nc.vector.tensor_scalar(
                mv[:, :, 1], mv[:, :, 1], n_dve, None, op0=mybir.AluOpType.mult
            )
            nc.vector.tensor_add(mv[:, :, 1], mv[:, :, 1], ssq_t[:])
            nc.vector.tensor_scalar(
                mv[:, :, 1], mv[:, :, 1], 1.0 / HW, None, op0=mybir.AluOpType.mult
            )
            ps_s = psA.tile([P, IB], F32, tag="ps")
            nc.tensor.matmul(
                ps_s[:8, : NCO * 2],
                aggm_sb[:],
                mv[:].rearrange("p co s -> p (co s)"),
                start=True, stop=True,
            )
            grp = stat.tile([8, NCO, 2], F32)
            nc.vector.tensor_copy(grp[:], ps_s[:8, : NCO * 2])
            g2 = stat.tile([8, NCO], F32)
            nc.vector.tensor_mul(g2[:], grp[:, :, 0], grp[:, :, 0])
            nc.vector.tensor_tensor(
                grp[:, :, 1], grp[:, :, 1], g2[:], mybir.AluOpType.subtract
            )  # var_g
            nc.scalar.activation(
                grp[:, :, 1], grp[:, :, 1], mybir.ActivationFunctionType.Sqrt,
                bias=eps_sb[:], scale=1.0,
            )
            nc.vector.reciprocal(grp[:, :, 1], grp[:, :, 1])  # rstd_g
            ps_b = psA.tile([P, IB], F32, tag="ps")
            nc.tensor.matmul(
                ps_b[:, : NCO * 2],
                bcm_sb[:],
                grp[:].rearrange("g co s -> g (co s)"),
                start=True, stop=True,
            )
            mvb = stat.tile([P, NCO, 2], F32)  # per-channel (mean_g, rstd_g)
            nc.vector.tensor_copy(mvb[:], ps_b[:, : NCO * 2])
            A = stat.tile([P, NCO], F32)
            nc.vector.tensor_mul(A[:], mvb[:, :, 1], gns_sb[:])
            t2 = stat.tile([P, NCO], F32)
            nc.vector.tensor_mul(t2[:], mvb[:, :, 0], A[:])
            Bc = stat.tile([P, NCO], F32)
            nc.vector.tensor_tensor(Bc[:], gnb_sb[:], t2[:], mybir.AluOpType.subtract)

            # ---------------- phase 2 prep: fold GN affine into weights
            # q/k/v = w @ (A*x + B) + b = (w.A) @ x + (w @ B + b); the
            # B-terms are per-output-channel constants computed with tiny
            # N=1 matmuls, then the big matmuls read x_bf directly.
            Bc_bf = stat.tile([P, NCO], BF16)
            nc.vector.tensor_copy(Bc_bf[:], Bc[:])
            kbias = stat.tile([P, NCO], F32)
            qbias = stat.tile([P, NCO], F32)
            for w_sb, b_sb, bias_col in (
                (wkt_sb, bk_sb, kbias),
                (wqt_sb, bq_sb, qbias),
            ):
                for o in range(NCO):
                    tps = psA.tile([P, IB], F32, tag="ps", name=f"tps_{o}")
                    for ci in range(NCO):
                        nc.tensor.matmul(
                            tps[:, 0:1],
                            w_sb[:, ci, o * P : (o + 1) * P],
                            Bc_bf[:, ci : ci + 1],
                            start=(ci == 0), stop=(ci == NCO - 1),
                        )
                    nc.vector.tensor_add(
                        bias_col[:, o : o + 1], tps[:, 0:1], b_sb[:, o : o + 1]
                    )
            # r[c] = B @ wvT, broadcast over partitions, + bv broadcast
            rps = psA.tile([P, IB], F32, tag="ps")
            for ci in range(NCO):
                nc.tensor.matmul(
                    rps[:1, :],
                    Bc_bf[:, ci : ci + 1],
                    wvt_sb[:, ci, :],
                    start=(ci == 0), stop=(ci == NCO - 1),
                )
            r_bf = stat.tile([1, C], BF16)
            nc.vector.tensor_copy(r_bf[:], rps[:1, :])
            vbps = psA.tile([P, IB], F32, tag="ps")
            nc.tensor.matmul(
                vbps[:, :], ones_bf[0:1, :], r_bf[:], start=True, stop=True
            )
            vbias = stat.tile([P, C], F32)
            nc.vector.tensor_add(vbias[:], vbps[:], bvb_sb[:])
            def scale_w(w_sb, name):
                # w' = w * A (per input channel = per partition), new tile so
                # the unscaled-weight bias matmuls don't serialize against it
                w_s = kqv.tile([P, NCO, C], FP8, name=name)
                for ci in range(NCO):
                    nc.vector.tensor_scalar_mul(
                        w_s[:, ci, :], w_sb[:, ci, :], A[:, ci : ci + 1]
                    )
                return w_s

            # ---------------- phase 2: q, then k, then vT from x8
            # Split outputs into per-region tiles so phase 4 pipelines into
            # phase 2 (exp(jg) only waits for the region it reads), and keep
            # ScalarE free of drain copies so its exp chain starts early.
            q_t = [kqv.tile([P, NCO, IB], FP8, name=f"q_t{i}") for i in range(NIB)]
            k_t = [kqv.tile([P, NCO, 2 * JBLK], FP8, name=f"k_t{i}") for i in range(4)]
            vT_t = [kqv.tile([P, 8, C], FP8, name=f"vT_t{i}") for i in range(4)]
            wqt_s = scale_w(wqt_sb, "wqt_s")
            for jb in range(NJB // 2):
                js, je = jb * JBLK, (jb + 1) * JBLK
                for o in range(NCO):
                    qps = psA.tile([P, IB], F32, tag="ps")
                    for cu in range(NCO // 2):
                        nc.tensor.matmul(
                            qps[:],
                            wqt_s[:, 2 * cu : 2 * cu + 2, o * P : (o + 1) * P],
                            x8_sb[:, 2 * cu : 2 * cu + 2, js:je],
                            start=(cu == 0), stop=(cu == NCO // 2 - 1),
                            perf_mode=mybir.MatmulPerfMode.DoubleRow,
                        )
                    if (jb + o) % 2 == 0:
                        nc.scalar.add(q_t[jb][:, o, :], qps[:], qbias[:, o : o + 1])
                    else:
                        nc.vector.tensor_scalar(
                            q_t[jb][:, o, :], qps[:], qbias[:, o : o + 1],
                            None, op0=mybir.AluOpType.add,
                        )
            wkt_s = scale_w(wkt_sb, "wkt_s")
            for jb in range(NJB):
                js, je = jb * JBLK, (jb + 1) * JBLK
                for o in range(NCO):
                    kps = psA.tile([P, IB], F32, tag="ps")
                    for cu in range(NCO // 2):
                        nc.tensor.matmul(
                            kps[:],
                            wkt_s[:, 2 * cu : 2 * cu + 2, o * P : (o + 1) * P],
                            x8_sb[:, 2 * cu : 2 * cu + 2, js:je],
                            start=(cu == 0), stop=(cu == NCO // 2 - 1),
                            perf_mode=mybir.MatmulPerfMode.DoubleRow,
                        )
                    kdst = k_t[jb // 2][:, o, (jb % 2) * JBLK : (jb % 2 + 1) * JBLK]
                    if (jb + o) % 2 == 0:
                        nc.scalar.add(kdst, kps[:], kbias[:, o : o + 1])
                    else:
                        nc.vector.tensor_scalar(
                            kdst, kps[:], kbias[:, o : o + 1],
                            None, op0=mybir.AluOpType.add,
                        )
            wvt_s = scale_w(wvt_sb, "wvt_s")
            for jb in range(NJB):
                js, je = jb * JBLK, (jb + 1) * JBLK
                for jc in range(JBLK // P):
                    vps = psA.tile([P, IB], F32, tag="ps")
                    for cu in range(NCO // 2):
                        nc.tensor.matmul(
                            vps[:],
                            x8_sb[:, 2 * cu : 2 * cu + 2, js + jc * P : js + (jc + 1) * P],
                            wvt_s[:, 2 * cu : 2 * cu + 2, :],
                            start=(cu == 0), stop=(cu == NCO // 2 - 1),
                            perf_mode=mybir.MatmulPerfMode.DoubleRow,
                        )
                    jg = jb * (JBLK // P) + jc
                    nc.vector.tensor_add(vT_t[jg // 8][:, jg % 8, :], vps[:], vbias[:])

            psq_ctx.__exit__(None, None, None)
            ps4_ctx = tc.tile_pool(name="psA", bufs=3, space="PSUM")
            psA = ps4_ctx.__enter__()
            psU_ctx = tc.tile_pool(name="psU", bufs=4, space="PSUM")
            psU = psU_ctx.__enter__()
            psD_ctx = tc.tile_pool(name="psD", bufs=1, space="PSUM")
            psD = psD_ctx.__enter__()

            # ---------------- phase 4: attention + proj + residual per i-block
            pending = []
            for ib in range(NIB):
                ibs, ibe = ib * IB, (ib + 1) * IB
                u_ps = [
                    psU.tile([P, IB], F32, tag="u", name=f"u_{ib}_{co}")
                    for co in range(NCO)
                ]
                d_ps = psD.tile([P, IB], F32, tag="d")

                NP2 = NJC // 2  # j-chunk pairs for fp8 DoubleRow

                def attnv(t, ex2):
                    # fp8 DoubleRow: one matmul contracts 256 j positions
                    for co in range(NCO):
                        nc.tensor.matmul(
                            u_ps[co],
                            vT_t[t // 4][:, 2 * (t % 4) : 2 * (t % 4) + 2, co * P : (co + 1) * P],
                            ex2[:],
                            start=(t == 0), stop=(t == NP2 - 1),
                            perf_mode=mybir.MatmulPerfMode.DoubleRow,
                        )
                    nc.tensor.matmul(
                        d_ps[:], ones8[:], ex2[:],
                        start=(t == 0), stop=(t == NP2 - 1),
                        perf_mode=mybir.MatmulPerfMode.DoubleRow,
                    )

                prev = None
                for t in range(NP2):
                    ex2 = expp.tile([P, 2, IB], FP8, tag="ex")
                    for r in range(2):
                        jg = 2 * t + r
                        sps = psA.tile([P, IB], F32, tag="ps")
                        for cu in range(NCO // 2):
                            nc.tensor.matmul(
                                sps[:],
                                k_t[jg // 8][:, 2 * cu : 2 * cu + 2,
                                             (jg % 8) * P : (jg % 8 + 1) * P],
                                q_t[ib][:, 2 * cu : 2 * cu + 2, :],
                                start=(cu == 0), stop=(cu == NCO // 2 - 1),
                                perf_mode=mybir.MatmulPerfMode.DoubleRow,
                            )
                        nc.scalar.activation(
                            ex2[:, r, :], sps[:], mybir.ActivationFunctionType.Exp,
                            bias=0.0, scale=SCALE,
                        )
                        if r == 0 and prev is not None:
                            attnv(*prev)
                            prev = None
                    prev = (t, ex2)
                attnv(*prev)

                u_sb = usb.tile([P, NCO, IB], BF16, tag="u_sb")
                for co in range(NCO):
                    nc.vector.tensor_copy(u_sb[:, co, :], u_ps[co])
                drec = drp.tile([P, IB], F32, tag="dr")
                nc.vector.reciprocal(drec[:], d_ps[:])
                x_blk = blk.tile([P, NCO, JBLK], F32, tag="xblk")
                nc.sync.dma_start(x_blk[:], x_r[:, :, ibs:ibe])
                for co in range(NCO):
                    nc.vector.tensor_scalar(
                        x_blk[:, co, :], x_blk[:, co, :], bp_sb[:, co : co + 1],
                        None, op0=mybir.AluOpType.add,
                    )

                def proj_epilogue(ibs=ibs, ibe=ibe, u_sb=u_sb, drec=drec, x_blk=x_blk):
                    out_sb = osb.tile([P, NCO, IB], F32, tag="out_sb")
                    for o in range(NCO):
                        pps = psA.tile([P, IB], F32, tag="ps", name=f"pps_{ibs}_{o}")
                        for ci in range(NCO):
                            nc.tensor.matmul(
                                pps[:],
                                wpt_sb[:, ci, o * P : (o + 1) * P],
                                u_sb[:, ci, :],
                                start=(ci == 0), stop=(ci == NCO - 1),
                            )
                        nc.vector.tensor_mul(out_sb[:, o, :], pps[:], drec[:])
                        nc.vector.tensor_add(
                            out_sb[:, o, :], out_sb[:, o, :], x_blk[:, o, :]
                        )
                        nc.sync.dma_start(out_r[:, o, ibs:ibe], out_sb[:, o, :])

                # defer this block's proj+epilogue until the next block's
                # attention loop is emitted so PE has ready work at the seam
                pending.append(proj_epilogue)
                if len(pending) > 1:
                    pending.pop(0)()
            for fn in pending:
                fn()
            psD_ctx.__exit__(None, None, None)
            psU_ctx.__exit__(None, None, None)
            ps4_ctx.__exit__(None, None, None)

    _split_multi_waits(nc)
    return nc


_NC_CACHE = []


def _get_nc():
    if not _NC_CACHE:
        _NC_CACHE.append(build_bass())
    return _NC_CACHE[0]


def _chunk_pc(v):
    """[512] per-channel vector -> [128, 4] (partition, chunk) layout."""
    return np.ascontiguousarray(v.reshape(NCO, P).T.astype(np.float32))


def kernel(x, gn_scale, gn_bias, wq, bq, wk, bk, wv, bv, wproj, bproj):
    x = np.asarray(x, dtype=np.float32)
    nc = _get_nc()

    aggm = np.zeros((P, 8), np.float32)
    for gg in range(8):
        aggm[gg * 16 : (gg + 1) * 16, gg] = 1.0 / 16.0
    bcm = np.zeros((8, P), np.float32)
    for gg in range(8):
        bcm[gg, gg * 16 : (gg + 1) * 16] = 1.0
    common = {
        "wqt": np.ascontiguousarray(np.asarray(wq, np.float32).T).astype(ml_dtypes.bfloat16),
        "wkt": np.ascontiguousarray(np.asarray(wk, np.float32).T).astype(ml_dtypes.bfloat16),
        "wvt": np.ascontiguousarray(np.asarray(wv, np.float32).T).astype(ml_dtypes.bfloat16),
        "wpt": np.ascontiguousarray(np.asarray(wproj, np.float32).T).astype(ml_dtypes.bfloat16),
        "bq": _chunk_pc(np.asarray(bq)),
        "bk": _chunk_pc(np.asarray(bk)),
        "bp": _chunk_pc(np.asarray(bproj)),
        "bvb": np.ascontiguousarray(np.tile(np.asarray(bv, np.float32)[None, :], (P, 1))),
        "gns": _chunk_pc(np.asarray(gn_scale)),
        "gnb": _chunk_pc(np.asarray(gn_bias)),
        "aggm": aggm,
        "bcm": bcm,
    }
    in_maps = []
    for r in range(8):
        s, h = r // 2, r % 2
        xs = x[s].reshape(C, HW)
        x_rot = np.ascontiguousarray(np.roll(xs, -h * IHALF, axis=1))
        in_maps.append({
            "x": x_rot,
            "xh": x_rot.astype(ml_dtypes.bfloat16),
            "x8": x_rot.astype(ml_dtypes.float8_e4m3),
            **common,
        })

    res = run_bass_kernel_spmd(nc, in_maps, core_ids=list(range(8)))

    out = np.empty((B, C, HW), np.float32)
    for r in range(8):
        s, h = r // 2, r % 2
        out[s][:, h * IHALF : (h + 1) * IHALF] = res.results[r]["out"]
    return out.reshape(B, C, H, W)



# revision 40
# speedup vs baseline: 2.1936x; 1.0224x over previous
"""AttnBlock (GroupNorm + single-head spatial attention + proj + residual)
on 8 Trainium2 NeuronCores via Bass/Tile.

Sharding: batch b=4 -> 4 samples x 2 cores each. Each core receives its
sample's x with its query-half columns rotated to the front (attention is
permutation-invariant over key positions), computes GroupNorm + k + v for
the full sample (redundant with its pair core) and q/attention/proj for its
2048 query positions. No cross-core communication.
"""

import numpy as np
import ml_dtypes

import concourse.bass as bass
import concourse.tile as tile
import concourse.mybir as mybir
from concourse.bass_utils import run_bass_kernel_spmd
from concourse.vector_clock import ScopedClock, VectorClock
from concourse.tile_scheduler import N_PROCS

# ---------------------------------------------------------------- constants
B, C, H, W = 4, 512, 64, 64
HW = H * W            # 4096
P = 128
NCO = C // P          # 4 channel chunks of 128
G = 32                # groups
IHALF = HW // 2       # 2048 query columns per core
IB = 512              # i-block width
NIB = IHALF // IB     # 4
JBLK = 512            # column block for GN/qkv phases
NJB = HW // JBLK      # 8
NJC = HW // P         # 32 j-chunks of 128
EPS = 1e-6
SCALE = float(1.0 / np.sqrt(C))
F32 = mybir.dt.float32
BF16 = mybir.dt.bfloat16
FP8 = mybir.dt.float8e4


# ------------------------------------------------- walrus single-wait fixes
class _TileContextFix(tile.TileContext):
    """TileContext whose tail drain splits sem waits across NOPs.

    The walrus build here rejects instructions carrying more than one sync
    wait ("Too many sync wait commands"), so the stock tail drain (one wait
    per outstanding proc) cannot codegen. Emit one single-wait NOP per proc
    before a wait-free drain.
    """

    def _drain_and_barrier(self, tick_clock, wait_clock):
        gc = tick_clock.global_clock
        for p in range(N_PROCS):
            if gc[p] == 0:
                continue
            partial = VectorClock([gc[q] if q == p else 0 for q in range(N_PROCS)])
            nop_inst = self.nc.sync.nop(nofuse=True, hint=f"tail_wait_{p}")
            wait_clock.add_sem_waits(nop_inst.ins, ScopedClock({None: partial}))
        self.nc.sync.drain()
        self.nc.all_engine_barrier()
        assert self.sems is not None
        popped = self.nc._tile_sem_poison_stack.pop()
        assert popped is self._sem_poison
        self.nc.clear_and_free_semaphores(list(self.sems.allocated().values()))


def _split_multi_waits(nc):
    """Split any instruction with N>1 sync waits into N-1 single-wait NOPs
    prepended on the same engine (same stream -> same ordering; sems are
    monotonic so waiting earlier is safe)."""
    fn = nc.m.functions[0]
    n_split = 0
    for bb in fn.blocks:
        insts = list(bb.instructions)
        out = []
        for inst in insts:
            si = inst.sync_info
            if si is not None and si.on_wait and len(si.on_wait) > 1:
                waits = list(si.on_wait)
                for w in waits[:-1]:
                    nop = mybir.InstNoOp(
                        name=nc.get_next_instruction_name(),
                        engine=inst.engine,
                        sync_info=mybir.SyncInfo(on_wait=[w], on_update=[]),
                        bass_nofuse=True,
                        ins=[],
                        outs=[],
                    )
                    out.append(nop)
                    n_split += 1
                inst.sync_info = mybir.SyncInfo(
                    on_wait=[waits[-1]], on_update=list(si.on_update or [])
                )
            out.append(inst)
        if len(out) != len(insts):
            bb.instructions[:] = out
    return n_split


# ------------------------------------------------------------- the kernel
def build_bass():
    nc = bass.Bass("TRN2", target_bir_lowering=False, debug=False, num_devices=8)

    x_d = nc.dram_tensor("x", [C, HW], F32, kind="ExternalInput")
    xh_d = nc.dram_tensor("xh", [C, HW], BF16, kind="ExternalInput")
    x8_d = nc.dram_tensor("x8", [C, HW], FP8, kind="ExternalInput")
    wqt_d = nc.dram_tensor("wqt", [C, C], BF16, kind="ExternalInput")
    wkt_d = nc.dram_tensor("wkt", [C, C], BF16, kind="ExternalInput")
    wvt_d = nc.dram_tensor("wvt", [C, C], BF16, kind="ExternalInput")
    wpt_d = nc.dram_tensor("wpt", [C, C], BF16, kind="ExternalInput")
    bq_d = nc.dram_tensor("bq", [P, NCO], F32, kind="ExternalInput")
    bk_d = nc.dram_tensor("bk", [P, NCO], F32, kind="ExternalInput")
    bp_d = nc.dram_tensor("bp", [P, NCO], F32, kind="ExternalInput")
    bvb_d = nc.dram_tensor("bvb", [P, C], F32, kind="ExternalInput")
    gns_d = nc.dram_tensor("gns", [P, NCO], F32, kind="ExternalInput")
    gnb_d = nc.dram_tensor("gnb", [P, NCO], F32, kind="ExternalInput")
    aggm_d = nc.dram_tensor("aggm", [P, 8], F32, kind="ExternalInput")
    bcm_d = nc.dram_tensor("bcm", [8, P], F32, kind="ExternalInput")
    out_d = nc.dram_tensor("out", [C, IHALF], F32, kind="ExternalOutput")

    x_r = x_d.ap().rearrange("(co p) j -> p co j", p=P)        # [128,4,4096]
    xh_r = xh_d.ap().rearrange("(co p) j -> p co j", p=P)
    x8_r = x8_d.ap().rearrange("(co p) j -> p co j", p=P)
    out_r = out_d.ap().rearrange("(co p) i -> p co i", p=P)    # [128,4,2048]

    with _TileContextFix(nc) as tc:
        with (
            tc.tile_pool(name="consts", bufs=1) as consts,
            tc.tile_pool(name="xbf", bufs=1) as xbf,
            tc.tile_pool(name="blk", bufs=3) as blk,
            tc.tile_pool(name="kqv", bufs=1) as kqv,
            tc.tile_pool(name="stat", bufs=1) as stat,
            tc.tile_pool(name="expp", bufs=6) as expp,
            tc.tile_pool(name="dram", bufs=1, space="DRAM") as dram,
            tc.tile_pool(name="usb", bufs=2) as usb,
            tc.tile_pool(name="drp", bufs=2) as drp,
            tc.tile_pool(name="osb", bufs=2) as osb,
        ):
            psq_ctx = tc.tile_pool(name="psQKV", bufs=6, space="PSUM")
            psA = psq_ctx.__enter__()

            # ---------------- phase 1 loads first (off the weight queues)
            x_bf = xbf.tile([P, NCO, HW], BF16)
            for jb in (6, 7, 0, 1, 2, 3, 4, 5):
                js, je = jb * JBLK, (jb + 1) * JBLK
                eng = nc.gpsimd if jb >= 6 else nc.sync
                eng.dma_start(x_bf[:, :, js:je], xh_r[:, :, js:je])
            x8_sb = xbf.tile([P, NCO, HW], FP8)
            nc.gpsimd.dma_start(x8_sb[:], x8_r)

            # ---------------- constants
            bq_sb = consts.tile([P, NCO], F32)
            nc.sync.dma_start(bq_sb[:], bq_d.ap())
            bk_sb = consts.tile([P, NCO], F32)
            nc.sync.dma_start(bk_sb[:], bk_d.ap())
            bp_sb = consts.tile([P, NCO], F32)
            nc.sync.dma_start(bp_sb[:], bp_d.ap())
            bvb_sb = consts.tile([P, C], F32)
            nc.sync.dma_start(bvb_sb[:], bvb_d.ap())
            gns_sb = consts.tile([P, NCO], F32)
            nc.sync.dma_start(gns_sb[:], gns_d.ap())
            gnb_sb = consts.tile([P, NCO], F32)
            nc.sync.dma_start(gnb_sb[:], gnb_d.ap())
            aggm_sb = consts.tile([P, 8], F32)
            nc.sync.dma_start(aggm_sb[:], aggm_d.ap())
            bcm_sb = consts.tile([8, P], F32)
            nc.sync.dma_start(bcm_sb[:], bcm_d.ap())
            wqt_sb = consts.tile([P, NCO, C], BF16)
            nc.sync.dma_start(wqt_sb[:], wqt_d.ap().rearrange("(ci p) o -> p ci o", p=P))
            wkt_sb = consts.tile([P, NCO, C], BF16)
            nc.sync.dma_start(wkt_sb[:], wkt_d.ap().rearrange("(ci p) o -> p ci o", p=P))
            wvt_sb = consts.tile([P, NCO, C], BF16)
            nc.sync.dma_start(wvt_sb[:], wvt_d.ap().rearrange("(ci p) o -> p ci o", p=P))
            wpt_sb = consts.tile([P, NCO, C], BF16)
            nc.sync.dma_start(wpt_sb[:], wpt_d.ap().rearrange("(ci p) o -> p ci o", p=P))
            ones_bf = consts.tile([P, P], BF16)
            nc.vector.memset(ones_bf[:], 1.0)
            ones8 = consts.tile([P, 2, P], FP8)
            nc.vector.memset(ones8[:], 1.0)
            eps_sb = consts.tile([8, 1], F32)
            nc.vector.memset(eps_sb[:], EPS)

            DVE_BLKS = [0, 1, 2, 3, 4, 5]
            ACT_BLKS = [6, 7]
            stats = stat.tile([P, NCO, len(DVE_BLKS), 6], F32)
            asum = stat.tile([P, NCO, 2, 2], F32)
            mv = stat.tile([P, NCO, 2], F32)

            # ---------------- phase 1: per-channel stats (DVE + ACT split)
            for bi, jb in enumerate(DVE_BLKS):
                js, je = jb * JBLK, (jb + 1) * JBLK
                for co in range(NCO):
                    nc.vector.bn_stats(stats[:, co, bi, :], x_bf[:, co, js:je])
            scr = stat.tile([P, JBLK], BF16)
            for bi, jb in enumerate(ACT_BLKS):
                js, je = jb * JBLK, (jb + 1) * JBLK
                for co in range(NCO):
                    nc.scalar.activation(
                        scr[:], x_bf[:, co, js:je],
                        mybir.ActivationFunctionType.Identity,
                        accum_out=asum[:, co, bi, 0:1],
                    )
                    nc.scalar.activation(
                        scr[:], x_bf[:, co, js:je],
                        mybir.ActivationFunctionType.Square,
                        accum_out=asum[:, co, bi, 1:2],
                    )

            # ---------------- phase 3: group stats -> per-channel affine A, B
            for co in range(NCO):
                nc.vector.bn_aggr(mv[:, co, :], stats[:, co, :, :])
            m2 = stat.tile([P, NCO], F32)
            nc.vector.tensor_mul(m2[:], mv[:, :, 0], mv[:, :, 0])
            nc.vector.tensor_add(mv[:, :, 1], mv[:, :, 1], m2[:])  # E[x^2] (DVE blocks)
            # merge ACT-block sums: stat = (stat6 * 3072 + act_sum) / 4096
            n_dve = float(len(DVE_BLKS) * JBLK)
            sum_t = stat.tile([P, NCO], F32)
            nc.vector.tensor_add(sum_t[:], asum[:, :, 0, 0], asum[:, :, 1, 0])
            ssq_t = stat.tile([P, NCO], F32)
            nc.vector.tensor_add(ssq_t[:], asum[:, :, 0, 1], asum[:, :, 1, 1])
            nc.vector.tensor_scalar(
                mv[:, :, 0], mv[:, :, 0], n_dve, None, op0=mybir.AluOpType.mult
            )
            nc.vector.tensor_add(mv[:, :, 0], mv[:, :, 0], sum_t[:])
            nc.vector.tensor_scalar(
                mv[:, :, 0], mv[:, :, 0], 1.0 / HW, None, op0=mybir.AluOpType.mult
            )
            nc.vector.tensor_scalar(
                mv[:, :, 1], mv[:, :, 1], n_dve, None, op0=mybir.AluOpType.mult
            )
            nc.vector.tensor_add(mv[:, :, 1], mv[:, :, 1], ssq_t[:])
            nc.vector.tensor_scalar(
                mv[:, :, 1], mv[:, :, 1], 1.0 / HW, None, op0=mybir.AluOpType.mult
            )
            ps_s = psA.tile([P, IB], F32, tag="ps")
            nc.tensor.matmul(
                ps_s[:8, : NCO * 2],
                aggm_sb[:],
                mv[:].rearrange("p co s -> p (co s)"),
                start=True, stop=True,
            )
            grp = stat.tile([8, NCO, 2], F32)
            nc.vector.tensor_copy(grp[:], ps_s[:8, : NCO * 2])
            g2 = stat.tile([8, NCO], F32)
            nc.vector.tensor_mul(g2[:], grp[:, :, 0], grp[:, :, 0])
            nc.vector.tensor_tensor(
                grp[:, :, 1], grp[:, :, 1], g2[:], mybir.AluOpType.subtract
            )  # var_g
            nc.scalar.activation(
                grp[:, :, 1], grp[:, :, 1], mybir.ActivationFunctionType.Sqrt,
                bias=eps_sb[:], scale=1.0,
            )
            nc.vector.reciprocal(grp[:, :, 1], grp[:, :, 1])  # rstd_g
            ps_b = psA.tile([P, IB], F32, tag="ps")
            nc.tensor.matmul(
                ps_b[:, : NCO * 2],
                bcm_sb[:],
                grp[:].rearrange("g co s -> g (co s)"),
                start=True, stop=True,
            )
            mvb = stat.tile([P, NCO, 2], F32)  # per-channel (mean_g, rstd_g)
            nc.vector.tensor_copy(mvb[:], ps_b[:, : NCO * 2])
            A = stat.tile([P, NCO], F32)
            nc.vector.tensor_mul(A[:], mvb[:, :, 1], gns_sb[:])
            t2 = stat.tile([P, NCO], F32)
            nc.vector.tensor_mul(t2[:], mvb[:, :, 0], A[:])
            Bc = stat.tile([P, NCO], F32)
            nc.vector.tensor_tensor(Bc[:], gnb_sb[:], t2[:], mybir.AluOpType.subtract)

            # ---------------- phase 2 prep: fold GN affine into weights
            # q/k/v = w @ (A*x + B) + b = (w.A) @ x + (w @ B + b); the
            # B-terms are per-output-channel constants computed with tiny
            # N=1 matmuls, then the big matmuls read x_bf directly.
            Bc_bf = stat.tile([P, NCO], BF16)
            nc.vector.tensor_copy(Bc_bf[:], Bc[:])
            kbias = stat.tile([P, NCO], F32)
            qbias = stat.tile([P, NCO], F32)
            for w_sb, b_sb, bias_col in (
                (wkt_sb, bk_sb, kbias),
                (wqt_sb, bq_sb, qbias),
            ):
                for o in range(NCO):
                    tps = psA.tile([P, IB], F32, tag="ps", name=f"tps_{o}")
                    for ci in range(NCO):
                        nc.tensor.matmul(
                            tps[:, 0:1],
                            w_sb[:, ci, o * P : (o + 1) * P],
                            Bc_bf[:, ci : ci + 1],
                            start=(ci == 0), stop=(ci == NCO - 1),
                        )
                    nc.vector.tensor_add(
                        bias_col[:, o : o + 1], tps[:, 0:1], b_sb[:, o : o + 1]
                    )
            # r[c] = B @ wvT, broadcast over partitions, + bv broadcast
            rps = psA.tile([P, IB], F32, tag="ps")
            for ci in range(NCO):
                nc.tensor.matmul(
                    rps[:1, :],
                    Bc_bf[:, ci : ci + 1],
                    wvt_sb[:, ci, :],
                    start=(ci == 0), stop=(ci == NCO - 1),
                )
            # s[c] = bv[c] + r[c] factors out of attention: U_biased = U_raw +
            # s*D, so (wp@U_biased)/D = (wp@U_raw)/D + wp@s -- fold wp@s into
            # the residual bias column instead of adding s to every v element.
            s_row = stat.tile([1, C], F32)
            nc.vector.tensor_add(s_row[:], rps[:1, :], bvb_sb[0:1, :])
            sd = dram.tile([C], F32)
            nc.sync.dma_start(sd[:].rearrange("(r c) -> r c", r=1), s_row[:])
            s_col = stat.tile([P, NCO], F32)
            nc.sync.dma_start(s_col[:], sd[:].rearrange("(co p) -> p co", p=P))
            s_col_bf = stat.tile([P, NCO], BF16)
            nc.vector.tensor_copy(s_col_bf[:], s_col[:])
            bp_eff = stat.tile([P, NCO], F32)
            for o in range(NCO):
                tps2 = psA.tile([P, IB], F32, tag="ps", name=f"tps2_{o}")
                for ci in range(NCO):
                    nc.tensor.matmul(
                        tps2[:, 0:1],
                        wpt_sb[:, ci, o * P : (o + 1) * P],
                        s_col_bf[:, ci : ci + 1],
                        start=(ci == 0), stop=(ci == NCO - 1),
                    )
                nc.vector.tensor_add(
                    bp_eff[:, o : o + 1], tps2[:, 0:1], bp_sb[:, o : o + 1]
                )
            def scale_w(w_sb, name):
                # w' = w * A (per input channel = per partition), new tile so
                # the unscaled-weight bias matmuls don't serialize against it
                w_s = kqv.tile([P, NCO, C], FP8, name=name)
                for ci in range(NCO):
                    nc.vector.tensor_scalar_mul(
                        w_s[:, ci, :], w_sb[:, ci, :], A[:, ci : ci + 1]
                    )
                return w_s

            # ---------------- phase 2: q, then k, then vT from x8
            # Split outputs into per-region tiles so phase 4 pipelines into
            # phase 2 (exp(jg) only waits for the region it reads), and keep
            # ScalarE free of drain copies so its exp chain starts early.
            q_t = [kqv.tile([P, NCO, IB], FP8, name=f"q_t{i}") for i in range(NIB)]
            k_t = [kqv.tile([P, NCO, 2 * JBLK], FP8, name=f"k_t{i}") for i in range(4)]
            vT_t = [kqv.tile([P, 8, C], FP8, name=f"vT_t{i}") for i in range(4)]
            wqt_s = scale_w(wqt_sb, "wqt_s")
            for jb in range(NJB // 2):
                js, je = jb * JBLK, (jb + 1) * JBLK
                for o in range(NCO):
                    qps = psA.tile([P, IB], F32, tag="ps")
                    for cu in range(NCO // 2):
                        nc.tensor.matmul(
                            qps[:],
                            wqt_s[:, 2 * cu : 2 * cu + 2, o * P : (o + 1) * P],
                            x8_sb[:, 2 * cu : 2 * cu + 2, js:je],
                            start=(cu == 0), stop=(cu == NCO // 2 - 1),
                            perf_mode=mybir.MatmulPerfMode.DoubleRow,
                        )
                    if (jb + o) % 2 == 0:
                        nc.scalar.add(q_t[jb][:, o, :], qps[:], qbias[:, o : o + 1])
                    else:
                        nc.vector.tensor_scalar(
                            q_t[jb][:, o, :], qps[:], qbias[:, o : o + 1],
                            None, op0=mybir.AluOpType.add,
                        )
            wkt_s = scale_w(wkt_sb, "wkt_s")
            for jb in range(NJB):
                js, je = jb * JBLK, (jb + 1) * JBLK
                for o in range(NCO):
                    kps = psA.tile([P, IB], F32, tag="ps")
                    for cu in range(NCO // 2):
                        nc.tensor.matmul(
                            kps[:],
                            wkt_s[:, 2 * cu : 2 * cu + 2, o * P : (o + 1) * P],
                            x8_sb[:, 2 * cu : 2 * cu + 2, js:je],
                            start=(cu == 0), stop=(cu == NCO // 2 - 1),
                            perf_mode=mybir.MatmulPerfMode.DoubleRow,
                        )
                    kdst = k_t[jb // 2][:, o, (jb % 2) * JBLK : (jb % 2 + 1) * JBLK]
                    if (jb + o) % 2 == 0:
                        nc.scalar.add(kdst, kps[:], kbias[:, o : o + 1])
                    else:
                        nc.vector.tensor_scalar(
                            kdst, kps[:], kbias[:, o : o + 1],
                            None, op0=mybir.AluOpType.add,
                        )
            wvt_s = scale_w(wvt_sb, "wvt_s")
            for jb in range(NJB):
                js, je = jb * JBLK, (jb + 1) * JBLK
                for jc in range(JBLK // P):
                    vps = psA.tile([P, IB], F32, tag="ps")
                    for cu in range(NCO // 2):
                        nc.tensor.matmul(
                            vps[:],
                            x8_sb[:, 2 * cu : 2 * cu + 2, js + jc * P : js + (jc + 1) * P],
                            wvt_s[:, 2 * cu : 2 * cu + 2, :],
                            start=(cu == 0), stop=(cu == NCO // 2 - 1),
                            perf_mode=mybir.MatmulPerfMode.DoubleRow,
                        )
                    jg = jb * (JBLK // P) + jc
                    if jg % 2 == 0:
                        nc.scalar.copy(vT_t[jg // 8][:, jg % 8, :], vps[:])
                    else:
                        nc.vector.tensor_copy(vT_t[jg // 8][:, jg % 8, :], vps[:])

            psq_ctx.__exit__(None, None, None)
            ps4_ctx = tc.tile_pool(name="psA", bufs=3, space="PSUM")
            psA = ps4_ctx.__enter__()
            psU_ctx = tc.tile_pool(name="psU", bufs=4, space="PSUM")
            psU = psU_ctx.__enter__()
            psD_ctx = tc.tile_pool(name="psD", bufs=1, space="PSUM")
            psD = psD_ctx.__enter__()

            # ---------------- phase 4: attention + proj + residual per i-block
            pending = []
            for ib in range(NIB):
                ibs, ibe = ib * IB, (ib + 1) * IB
                u_ps = [
                    psU.tile([P, IB], F32, tag="u", name=f"u_{ib}_{co}")
                    for co in range(NCO)
                ]
                d_ps = psD.tile([P, IB], F32, tag="d")

                NP2 = NJC // 2  # j-chunk pairs for fp8 DoubleRow

                def attnv(t, ex2):
                    # fp8 DoubleRow: one matmul contracts 256 j positions
                    for co in range(NCO):
                        nc.tensor.matmul(
                            u_ps[co],
                            vT_t[t // 4][:, 2 * (t % 4) : 2 * (t % 4) + 2, co * P : (co + 1) * P],
                            ex2[:],
                            start=(t == 0), stop=(t == NP2 - 1),
                            perf_mode=mybir.MatmulPerfMode.DoubleRow,
                        )
                    nc.tensor.matmul(
                        d_ps[:], ones8[:], ex2[:],
                        start=(t == 0), stop=(t == NP2 - 1),
                        perf_mode=mybir.MatmulPerfMode.DoubleRow,
                    )

                prev = None
                for t in range(NP2):
                    ex2 = expp.tile([P, 2, IB], FP8, tag="ex")
                    for r in range(2):
                        jg = 2 * t + r
                        sps = psA.tile([P, IB], F32, tag="ps")
                        for cu in range(NCO // 2):
                            nc.tensor.matmul(
                                sps[:],
                                k_t[jg // 8][:, 2 * cu : 2 * cu + 2,
                                             (jg % 8) * P : (jg % 8 + 1) * P],
                                q_t[ib][:, 2 * cu : 2 * cu + 2, :],
                                start=(cu == 0), stop=(cu == NCO // 2 - 1),
                                perf_mode=mybir.MatmulPerfMode.DoubleRow,
                            )
                        nc.scalar.activation(
                            ex2[:, r, :], sps[:], mybir.ActivationFunctionType.Exp,
                            bias=0.0, scale=SCALE,
                        )
                        if r == 0 and prev is not None:
                            attnv(*prev)
                            prev = None
                    prev = (t, ex2)
                attnv(*prev)

                u_sb = usb.tile([P, NCO, IB], BF16, tag="u_sb")
                for co in range(NCO):
                    nc.vector.tensor_copy(u_sb[:, co, :], u_ps[co])
                drec = drp.tile([P, IB], F32, tag="dr")
                nc.vector.reciprocal(drec[:], d_ps[:])
                x_blk = blk.tile([P, NCO, JBLK], F32, tag="xblk")
                nc.sync.dma_start(x_blk[:], x_r[:, :, ibs:ibe])
                for co in range(NCO):
                    nc.vector.tensor_scalar(
                        x_blk[:, co, :], x_blk[:, co, :], bp_eff[:, co : co + 1],
                        None, op0=mybir.AluOpType.add,
                    )

                def proj_epilogue(ibs=ibs, ibe=ibe, u_sb=u_sb, drec=drec, x_blk=x_blk):
                    out_sb = osb.tile([P, NCO, IB], F32, tag="out_sb")
                    for o in range(NCO):
                        pps = psA.tile([P, IB], F32, tag="ps", name=f"pps_{ibs}_{o}")
                        for ci in range(NCO):
                            nc.tensor.matmul(
                                pps[:],
                                wpt_sb[:, ci, o * P : (o + 1) * P],
                                u_sb[:, ci, :],
                                start=(ci == 0), stop=(ci == NCO - 1),
                            )
                        nc.vector.tensor_mul(out_sb[:, o, :], pps[:], drec[:])
                        nc.vector.tensor_add(
                            out_sb[:, o, :], out_sb[:, o, :], x_blk[:, o, :]
                        )
                        nc.sync.dma_start(out_r[:, o, ibs:ibe], out_sb[:, o, :])

                # defer this block's proj+epilogue until the next block's
                # attention loop is emitted so PE has ready work at the seam
                pending.append(proj_epilogue)
                if len(pending) > 1:
                    pending.pop(0)()
            for fn in pending:
                fn()
            psD_ctx.__exit__(None, None, None)
            psU_ctx.__exit__(None, None, None)
            ps4_ctx.__exit__(None, None, None)

    _split_multi_waits(nc)
    return nc


_NC_CACHE = []


def _get_nc():
    if not _NC_CACHE:
        _NC_CACHE.append(build_bass())
    return _NC_CACHE[0]


def _chunk_pc(v):
    """[512] per-channel vector -> [128, 4] (partition, chunk) layout."""
    return np.ascontiguousarray(v.reshape(NCO, P).T.astype(np.float32))


def kernel(x, gn_scale, gn_bias, wq, bq, wk, bk, wv, bv, wproj, bproj):
    x = np.asarray(x, dtype=np.float32)
    nc = _get_nc()

    aggm = np.zeros((P, 8), np.float32)
    for gg in range(8):
        aggm[gg * 16 : (gg + 1) * 16, gg] = 1.0 / 16.0
    bcm = np.zeros((8, P), np.float32)
    for gg in range(8):
        bcm[gg, gg * 16 : (gg + 1) * 16] = 1.0
    common = {
        "wqt": np.ascontiguousarray(np.asarray(wq, np.float32).T).astype(ml_dtypes.bfloat16),
        "wkt": np.ascontiguousarray(np.asarray(wk, np.float32).T).astype(ml_dtypes.bfloat16),
        "wvt": np.ascontiguousarray(np.asarray(wv, np.float32).T).astype(ml_dtypes.bfloat16),
        "wpt": np.ascontiguousarray(np.asarray(wproj, np.float32).T).astype(ml_dtypes.bfloat16),
        "bq": _chunk_pc(np.asarray(bq)),
        "bk": _chunk_pc(np.asarray(bk)),
        "bp": _chunk_pc(np.asarray(bproj)),
        "bvb": np.ascontiguousarray(np.tile(np.asarray(bv, np.float32)[None, :], (P, 1))),
        "gns": _chunk_pc(np.asarray(gn_scale)),
        "gnb": _chunk_pc(np.asarray(gn_bias)),
        "aggm": aggm,
        "bcm": bcm,
    }
    in_maps = []
    for r in range(8):
        s, h = r // 2, r % 2
        xs = x[s].reshape(C, HW)
        x_rot = np.ascontiguousarray(np.roll(xs, -h * IHALF, axis=1))
        in_maps.append({
            "x": x_rot,
            "xh": x_rot.astype(ml_dtypes.bfloat16),
            "x8": x_rot.astype(ml_dtypes.float8_e4m3),
            **common,
        })

    res = run_bass_kernel_spmd(nc, in_maps, core_ids=list(range(8)))

    out = np.empty((B, C, HW), np.float32)
    for r in range(8):
        s, h = r // 2, r % 2
        out[s][:, h * IHALF : (h + 1) * IHALF] = res.results[r]["out"]
    return out.reshape(B, C, H, W)

